# revision 1
# baseline (speedup 1.0000x reference)
"""Trainium2 Bass kernel for the MoE routing module (nn_MoE_53042846105633).

Strategy: dense expert-parallel across 8 NeuronCores. Core e computes
expert e's MLP over ALL tokens (top-k masked-dense math, identical to the
reference), weights its output by that expert's routing weight (0 for
tokens that didn't pick it in top-2), and the host sums the 8 partials.

Router precision: top-2 selection must match an fp32 reference (min
top2/top3 logit gap on this data is 6e-4, far below single-bf16 error),
so both router matmuls run as bf16x2 (hi/lo split: W_h@x_h + W_h@x_l +
W_l@x_h accumulated in fp32 PSUM; logit error ~1e-5). The expert MLP
runs in plain bf16 (fp32 accumulate), giving ~3e-3 relative output error.

Timeline per core (~234us median, +-3us): the 768 expert matmuls (16
hid-tiles x 2 token-tiles x 24 K-chunks, N=512, ~216ns each warm) stream
weights from HBM;
the replicated router is emitted mid-loop so its DVE/ACT top-2 chains
hide under expert matmul work; a small N=10 matmul + per-token weight
multiply finishes. Collectives are deliberately avoided: a NEFF with
collectives runs the PE at 2.0 GHz instead of 2.4 (P0 power profile),
costing far more than the AllGather would save.
"""

import sys

sys.path.insert(0, "/opt/trn_rl_repo")

import numpy as np
import ml_dtypes

BF16 = ml_dtypes.bfloat16

# Model dims (fixed for this problem)
B = 1024          # tokens
DIN = 3072        # input features
RHID = 128        # router hidden
E = 8             # experts = cores
EHID = 2048       # expert hidden
NCLS = 10         # classes
KC1 = DIN // 128  # 24 K-chunks for DIN contraction
KC2 = EHID // 128 # 16 K-chunks for EHID contraction
MT = B // 128     # 8 token tiles
NT = B // 512     # 2 N-tiles of 512 tokens

_PROGRAM = None
LAST_RESULTS = None


def _ensure_axon_profile_hook():
    """bass_utils' trace=True path imports antenv.axon_hooks, which this
    image lacks. Provide it (backed by libaxon_pjrt.so's NRT profile C API)
    so NTFF profiling works; degrade silently if unavailable."""
    import contextlib
    import ctypes
    import os
    import types

    try:
        from antenv.axon_hooks import get_axon_ntff_profile_hook  # noqa: F401
        return
    except ImportError:
        pass
    try:
        import antenv
    except ImportError:
        return

    state = {"hook": None}
    mod = types.ModuleType("antenv.axon_hooks")
    mod.set_axon_ntff_profile_hook = lambda h: state.__setitem__("hook", h)
    mod.get_axon_ntff_profile_hook = lambda: state["hook"]
    sys.modules["antenv.axon_hooks"] = mod
    antenv.axon_hooks = mod

    so_path = "/opt/axon/libaxon_pjrt.so"
    if not os.path.exists(so_path):
        return
    try:
        lib = ctypes.CDLL(so_path)
    except OSError:
        return
    if not hasattr(lib, "axon_start_nrt_profile"):
        return
    lib.axon_start_nrt_profile.argtypes = [
        ctypes.POINTER(ctypes.c_int64), ctypes.c_size_t]
    lib.axon_start_nrt_profile.restype = ctypes.c_int64
    lib.axon_stop_nrt_profile.argtypes = [ctypes.c_char_p]
    lib.axon_stop_nrt_profile.restype = ctypes.c_int64

    @contextlib.contextmanager
    def _hook(output_dir, device_ids):
        import jax

        jax.devices()
        if device_ids:
            ids = (ctypes.c_int64 * len(device_ids))(*device_ids)
            rc = lib.axon_start_nrt_profile(ids, len(device_ids))
        else:
            rc = lib.axon_start_nrt_profile(None, 0)
        if rc != 0:
            raise RuntimeError(f"axon_start_nrt_profile rc={rc}")
        try:
            yield
        finally:
            n = lib.axon_stop_nrt_profile(str(output_dir).encode())
            print(f"profile: {n} ntff file(s) -> {output_dir}",
                  file=sys.stderr)

    state["hook"] = _hook


def _build_program():
    import concourse.tile as tile
    from concourse import bacc, mybir

    f32 = mybir.dt.float32
    bf = mybir.dt.bfloat16
    AF = mybir.ActivationFunctionType
    ALU = mybir.AluOpType

    # Bacc (not raw Bass): its compile() pass splits multi-sem waits onto
    # EventSemaphore instructions (TRN2 allows 1 wait per instruction).
    nc = bacc.Bacc("TRN2", debug=False, num_devices=E)

    # ---- DRAM I/O ----------------------------------------------------------
    # x (hi/lo bf16 split), layout [i, k, n]: element = xf[n, 128k + i]
    d_xh = nc.dram_tensor("xh", [128, KC1, B], bf, kind="ExternalInput")
    d_xl = nc.dram_tensor("xl", [128, KC1, B], bf, kind="ExternalInput")
    # router W1 (hi/lo), layout [i, k, j]: element = rW1[128k + i, j]
    d_w1h = nc.dram_tensor("w1h", [128, KC1, RHID], bf, kind="ExternalInput")
    d_w1l = nc.dram_tensor("w1l", [128, KC1, RHID], bf, kind="ExternalInput")
    d_rw2h = nc.dram_tensor("rw2h", [RHID, E], bf, kind="ExternalInput")
    d_rw2l = nc.dram_tensor("rw2l", [RHID, E], bf, kind="ExternalInput")
    d_rb1 = nc.dram_tensor("rb1", [RHID, 1], f32, kind="ExternalInput")
    # rb2/eb2 pre-tiled to 128 partitions (biases vary along the free dim,
    # so they fold into DVE adds instead of K=1 broadcast matmuls)
    d_rb2t = nc.dram_tensor("rb2t", [128, E], f32, kind="ExternalInput")
    # expert weights for this core's expert
    # ew1 layout [m, i, (k j)]: element = eW1[e][128k + i, 128m + j]
    d_ew1 = nc.dram_tensor("ew1", [KC2, 128, DIN], bf, kind="ExternalInput")
    # ew2 layout [i, k2, c]: element = eW2[e][128*k2 + i, c]
    d_ew2 = nc.dram_tensor("ew2", [128, KC2, NCLS], bf, kind="ExternalInput")
    # eb1 layout [i, m]: element = eb1[e][128m + i]
    d_eb1 = nc.dram_tensor("eb1", [128, KC2], f32, kind="ExternalInput")
    d_eb2t = nc.dram_tensor("eb2t", [128, NCLS], f32, kind="ExternalInput")
    # one-hot row for this core's expert, tiled to 128 partitions
    d_sel = nc.dram_tensor("sel", [128, E], f32, kind="ExternalInput")
    # weighted partial output (host sums over cores)
    d_out = nc.dram_tensor("out", [B, NCLS], f32, kind="ExternalOutput")

    with tile.TileContext(nc) as tc:
        with (
            tc.tile_pool(name="const", bufs=1) as cp,
            tc.tile_pool(name="wstream", bufs=6) as wp,
            tc.tile_pool(name="psum", bufs=1, space="PSUM") as pp,
            tc.tile_pool(name="outp", bufs=1) as op,
        ):
            # ---- HAM pre-warm ----------------------------------------------
            # The first input chunks can't arrive before ~12us (DMA init +
            # transfer + scheduling), so the PE would start cold (1.2 GHz).
            # Full-array K=128 dummies during that window flip the HAM clock
            # gate to 2.4 GHz for free (small-K matmuls don't count as
            # PE-busy; DVE memset, not gpsimd — its library load takes ~6us).
            warmt = cp.tile([128, 128], bf, tag="warmt", name="warmt")
            nc.vector.memset(warmt[:], 1.0)
            warm = pp.tile([128, 128], f32, tag="po", bufs=2, name="warm")
            for _i in range(44):
                nc.tensor.matmul(warm[:], warmt[:], warmt[:],
                                 start=True, stop=True)

            # ---- input DMA (emission order ~= DMA queue order) -------------
            # Critical path: mm1 needs xk[0] + the first half of ew1[0]
            # immediately; everything the router needs can trickle in later.
            wts = {}

            def load_ew1(m):
                # two half-DMAs: with subtile deps the k=0 matmul only waits
                # for the first half (smaller time-to-first-matmul)
                wt = wp.tile([128, DIN], bf, tag="ew1", name=f"ew1m{m}")
                nc.sync.dma_start(wt[:, :DIN // 2], d_ew1[m][:, :DIN // 2])
                nc.sync.dma_start(wt[:, DIN // 2:], d_ew1[m][:, DIN // 2:])
                wts[m] = wt

            xk = []
            for k in range(KC1):
                t = cp.tile([128, B], bf, tag=f"xk{k}", name=f"xk{k}")
                xk.append(t)
            nc.sync.dma_start(xk[0][:], d_xh[:, 0, :])
            load_ew1(0)
            for k in range(1, KC1):
                nc.sync.dma_start(xk[k][:], d_xh[:, k, :])
            for _m in range(1, 6):
                load_ew1(_m)
            eb1t = cp.tile([128, KC2], f32, tag="eb1", name="eb1t")
            nc.sync.dma_start(eb1t[:], d_eb1[:])
            rb1t = cp.tile([RHID, 1], f32, tag="rb1", name="rb1t")
            nc.sync.dma_start(rb1t[:], d_rb1[:])
            w1ht = cp.tile([128, KC1, RHID], bf, tag="w1h", name="w1ht")
            nc.sync.dma_start(w1ht[:], d_w1h[:])
            w1lt = cp.tile([128, KC1, RHID], bf, tag="w1l", name="w1lt")
            nc.sync.dma_start(w1lt[:], d_w1l[:])
            xlk = []
            for k in range(KC1):
                t = cp.tile([128, B], bf, tag=f"xlk{k}", name=f"xlk{k}")
                nc.sync.dma_start(t[:], d_xl[:, k, :])
                xlk.append(t)
            ew2t = cp.tile([128, KC2, NCLS], bf, tag="ew2", name="ew2t")
            nc.sync.dma_start(ew2t[:], d_ew2[:])
            eb2t = cp.tile([128, NCLS], f32, tag="eb2", name="eb2t")
            nc.sync.dma_start(eb2t[:], d_eb2t[:])
            selt = cp.tile([128, E], f32, tag="sel", name="selt")
            nc.sync.dma_start(selt[:], d_sel[:])
            rw2ht = cp.tile([RHID, E], bf, tag="rw2h", name="rw2ht")
            nc.sync.dma_start(rw2ht[:], d_rw2h[:])
            rw2lt = cp.tile([RHID, E], bf, tag="rw2l", name="rw2lt")
            nc.sync.dma_start(rw2lt[:], d_rw2l[:])
            rb2t = cp.tile([128, E], f32, tag="rb2t", name="rb2t")
            nc.sync.dma_start(rb2t[:], d_rb2t[:])

            # ehT: relu(eW1.T @ x) in [hid, tok] layout, bf16 — one tile per
            # 128-hid chunk so mm2's k2=0 matmuls depend only on their chunk
            ehs = [cp.tile([128, B], bf, tag=f"eh{m}", name=f"eh{m}")
                   for m in range(KC2)]

            wmy = cp.tile([128, MT], f32, tag="wmy", name="wmy")

            def emit_router():
                # ---- replicated router (all tokens, bf16x2) ---------------
                # Emitted mid-way through mm1 so its DVE/ACT top-2 chains
                # hide under the remaining expert matmul work.
                rh = cp.tile([RHID, B], f32, tag="rh", name="rh")
                for n in range(NT):
                    psr = pp.tile([128, 512], f32, tag="mm1", bufs=4,
                                  name=f"psr{n}")
                    passes = [(w1ht, xk), (w1ht, xlk), (w1lt, xk)]
                    for pi, (wt_, xs_) in enumerate(passes):
                        for k in range(KC1):
                            nc.tensor.matmul(
                                psr[:],
                                wt_[:, k, :],
                                xs_[k][:, n * 512:(n + 1) * 512],
                                start=(pi == 0 and k == 0),
                                stop=(pi == 2 and k == KC1 - 1),
                            )
                    nc.scalar.activation(
                        rh[:, n * 512:(n + 1) * 512], psr[:],
                        AF.Relu, bias=rb1t[:, 0:1],
                    )
                # hi/lo split of rh so the logit matmul is bf16x2 too (keeps
                # fp32 matmuls off PE entirely)
                rhh = cp.tile([RHID, B], bf, tag="rhh", name="rhh")
                nc.vector.tensor_copy(rhh[:], rh[:])
                rhl = cp.tile([RHID, B], bf, tag="rhl", name="rhl")
                nc.vector.tensor_sub(rhl[:], rh[:], rhh[:])

                # logits + top-2 weight per token tile; for expert e:
                #   w = exp(l_e - m1) * (l_e >= t2) / (1 + exp(t2 - m1))
                for mt in range(MT):
                    tsl = slice(mt * 128, (mt + 1) * 128)
                    pl = pp.tile([128, E], f32, tag="lg", bufs=2,
                                 name=f"pl{mt}")
                    nc.tensor.matmul(pl[:], rhh[:, tsl], rw2ht[:],
                                     start=True, stop=False)
                    nc.tensor.matmul(pl[:], rhh[:, tsl], rw2lt[:],
                                     start=False, stop=False)
                    nc.tensor.matmul(pl[:], rhl[:, tsl], rw2ht[:],
                                     start=False, stop=True)
                    lg = op.tile([128, E], f32, tag="lg_sb", bufs=2,
                                 name=f"lg{mt}")
                    # psum -> sbuf copy fused with the fp32 rb2 bias add
                    nc.vector.tensor_add(lg[:], pl[:], rb2t[:])
                    m1 = op.tile([128, 1], f32, tag="m1", bufs=2,
                                 name=f"m1_{mt}")
                    nc.vector.reduce_max(m1[:], lg[:],
                                         axis=mybir.AxisListType.X)
                    nm1 = op.tile([128, 1], f32, tag="nm1", bufs=2,
                                  name=f"nm1_{mt}")
                    nc.vector.tensor_scalar_mul(nm1[:], m1[:], -1.0)
                    ismax = op.tile([128, E], f32, tag="ismax", bufs=2,
                                    name=f"ismax{mt}")
                    nc.vector.tensor_scalar(ismax[:], lg[:], m1[:], None,
                                            ALU.is_ge)
                    nc.vector.tensor_scalar_mul(ismax[:], ismax[:], -1e30)
                    nc.vector.tensor_add(ismax[:], ismax[:], lg[:])
                    t2 = op.tile([128, 1], f32, tag="t2", bufs=2,
                                 name=f"t2_{mt}")
                    nc.vector.reduce_max(t2[:], ismax[:],
                                         axis=mybir.AxisListType.X)
                    w_all = op.tile([128, E], f32, tag="w_all", bufs=2,
                                    name=f"w_all{mt}")
                    nc.vector.tensor_scalar(w_all[:], lg[:], t2[:], None,
                                            ALU.is_ge)
                    enum = op.tile([128, E], f32, tag="enum", bufs=2,
                                   name=f"enum{mt}")
                    nc.scalar.activation(enum[:], lg[:], AF.Exp,
                                         bias=nm1[:, 0:1])
                    den = op.tile([128, 1], f32, tag="den", bufs=2,
                                  name=f"den{mt}")
                    nc.scalar.activation(den[:], t2[:], AF.Exp,
                                         bias=nm1[:, 0:1])
                    nc.vector.tensor_scalar_add(den[:], den[:], 1.0)
                    rden = op.tile([128, 1], f32, tag="rden", bufs=2,
                                   name=f"rden{mt}")
                    nc.vector.reciprocal(rden[:], den[:])
                    nc.vector.tensor_mul(w_all[:], w_all[:], enum[:])
                    nc.vector.tensor_mul(w_all[:], w_all[:], selt[:])
                    wn = op.tile([128, 1], f32, tag="wn", bufs=2,
                                 name=f"wn{mt}")
                    nc.vector.reduce_sum(wn[:], w_all[:],
                                         axis=mybir.AxisListType.X)
                    nc.vector.tensor_scalar(
                        wmy[:, mt:mt + 1], wn[:], rden[:], None, ALU.mult)

            # ---- expert matmul 1: ehT[m] = relu(eW1[:, m-tile].T @ x + b) --
            for m in range(KC2):
                if m == 5:
                    emit_router()
                wt = wts[m]
                if m + 6 < KC2:
                    load_ew1(m + 6)
                pss = [pp.tile([128, 512], f32, tag="mm1", bufs=4,
                               name=f"ps1_{m}_{n}") for n in range(NT)]
                if m < 2:
                    # k-outer during the DMA ramp: consumes each x chunk at
                    # half the rate, keeping matmuls at pace with x arrival
                    for k in range(KC1):
                        for n in range(NT):
                            nc.tensor.matmul(
                                pss[n][:],
                                wt[:, k * 128:(k + 1) * 128],
                                xk[k][:, n * 512:(n + 1) * 512],
                                start=(k == 0),
                                stop=(k == KC1 - 1),
                            )
                else:
                    for n in range(NT):
                        for k in range(KC1):
                            nc.tensor.matmul(
                                pss[n][:],
                                wt[:, k * 128:(k + 1) * 128],
                                xk[k][:, n * 512:(n + 1) * 512],
                                start=(k == 0),
                                stop=(k == KC1 - 1),
                            )
                for n in range(NT):
                    nc.scalar.activation(
                        ehs[m][:, n * 512:(n + 1) * 512], pss[n][:],
                        AF.Relu, bias=eb1t[:, m:m + 1],
                    )

            # ---- expert matmul 2 + weighted combine ------------------------
            for mt in range(MT):
                po = pp.tile([128, NCLS], f32, tag="po", bufs=2, name=f"po{mt}")
                for k2 in range(KC2):
                    nc.tensor.matmul(
                        po[:],
                        ehs[k2][:, mt * 128:(mt + 1) * 128],
                        ew2t[:, k2, :],
                        start=(k2 == 0),
                        stop=(k2 == KC2 - 1),
                    )
                osb = op.tile([128, NCLS], f32, tag="osb", bufs=3,
                              name=f"osb{mt}")
                nc.vector.tensor_add(osb[:], po[:], eb2t[:])
                nc.vector.tensor_scalar(
                    osb[:], osb[:], wmy[:, mt:mt + 1], None, ALU.mult,
                )
                nc.sync.dma_start(d_out[mt * 128:(mt + 1) * 128, :], osb[:])

    return nc


def _get_program():
    global _PROGRAM
    if _PROGRAM is None:
        _PROGRAM = _build_program()
    return _PROGRAM


def _prep_inputs(x, rW1, rb1, rW2, rb2, eW1, eb1, eW2, eb2):
    """Host-side shard/layout prep. Returns in_maps for the 8 cores."""
    xf = np.ascontiguousarray(x.reshape(B, DIN), dtype=np.float32)
    # [i, k, n] layout with hi/lo bf16 split
    xt = xf.reshape(B, KC1, 128).transpose(2, 1, 0)
    xh = xt.astype(BF16)
    xl = (xt - xh.astype(np.float32)).astype(BF16)
    xh = np.ascontiguousarray(xh)
    xl = np.ascontiguousarray(xl)

    w1 = np.asarray(rW1, np.float32).reshape(KC1, 128, RHID).transpose(1, 0, 2)
    w1h = w1.astype(BF16)
    w1l = (w1 - w1h.astype(np.float32)).astype(BF16)
    w1h = np.ascontiguousarray(w1h)
    w1l = np.ascontiguousarray(w1l)

    rw2f = np.asarray(rW2, np.float32)
    rw2h = rw2f.astype(BF16)
    rw2l = np.ascontiguousarray((rw2f - rw2h.astype(np.float32)).astype(BF16))
    rw2h = np.ascontiguousarray(rw2h)
    rb1c = np.ascontiguousarray(np.asarray(rb1, np.float32).reshape(RHID, 1))
    rb2t = np.ascontiguousarray(
        np.tile(np.asarray(rb2, np.float32).reshape(1, E), (128, 1)))

    in_maps = []
    for e in range(E):
        ew1 = np.ascontiguousarray(
            np.asarray(eW1[e], np.float32)
            .reshape(KC1, 128, KC2, 128)
            .transpose(2, 1, 0, 3)
            .reshape(KC2, 128, DIN)
            .astype(BF16)
        )
        ew2 = np.ascontiguousarray(
            np.asarray(eW2[e], np.float32)
            .reshape(KC2, 128, NCLS)
            .transpose(1, 0, 2)
            .astype(BF16)
        )
        eb1t = np.ascontiguousarray(
            np.asarray(eb1[e], np.float32).reshape(KC2, 128).T
        )
        eb2r = np.ascontiguousarray(
            np.tile(np.asarray(eb2[e], np.float32).reshape(1, NCLS), (128, 1))
        )
        sel = np.zeros((128, E), np.float32)
        sel[:, e] = 1.0
        in_maps.append({
            "xh": xh, "xl": xl,
            "w1h": w1h, "w1l": w1l,
            "rw2h": rw2h, "rw2l": rw2l, "rb1": rb1c, "rb2t": rb2t,
            "ew1": ew1, "ew2": ew2, "eb1": eb1t, "eb2t": eb2r,
            "sel": sel,
        })
    return in_maps


def kernel(x, rW1, rb1, rW2, rb2, eW1, eb1, eW2, eb2):
    global LAST_RESULTS
    _ensure_axon_profile_hook()
    from concourse.bass_utils import run_bass_kernel_spmd

    nc = _get_program()
    if not nc.is_finalized():
        # bass2jax serializes the module as-is; Bacc's lowering passes
        # (register alloc, wait splitting) only run in finalize().
        nc.finalize()
    in_maps = _prep_inputs(x, rW1, rb1, rW2, rb2, eW1, eb1, eW2, eb2)
    res = run_bass_kernel_spmd(nc, in_maps, core_ids=list(range(E)))
    LAST_RESULTS = res
    out = np.zeros((B, NCLS), np.float32)
    for r in res.results:
        out += np.asarray(r["out"], np.float32)
    return out



# revision 2
# speedup vs baseline: 2.3651x; 2.3651x over previous
"""Trainium2 Bass kernel for the MoE routing module (nn_MoE_53042846105633).

Strategy: top-2 sparse dispatch, expert-parallel across 8 NeuronCores.
The router (0.8% of reference FLOPs) runs on host in fp64 during input
sharding -- the sharding_hint's "dispatch of tokens by top-k expert id"
-- which cuts device matmul work 4x vs dense: only the 2048 routed
(token, expert) pairs are computed instead of 8 experts x 1024 tokens.

Token counts per expert are skewed (e.g. [199,263,548,380,142,99,271,146]
for the seed-0 data), so each expert's 16 hidden tiles are split into two
8-tile halves and the 16 halves are packed onto cores as (big, small)
slot pairs: slot0 capacity C0 = largest expert count, slot1 capacity C1 =
5th-largest. All 8 cores run the identical instruction stream (SPMD);
per-core data (which expert half + which token columns) differs. With 16
half-slots and >=2 slots per expert this (C0 + C1) packing is optimal.

Per core: mm1 = 8 hid-tiles x 24 K-chunks over C0 cols + 8 x 24 over C1
(~60us PE at bf16), relu -> eh bf16, mm2 contracts eh against eW2 per
128-token stationary slice, bias + per-token routing weight (0 on padding)
-> weighted partial rows DMA'd out; host scatter-adds them into [B, 10].
Weights stream from HBM (12.6MB/core) well under the mm1 time. No
collectives (a NEFF with collectives drops the PE to 2.0 GHz). Expert
math is plain bf16 (fp32 PSUM accumulate), ~3e-3 relative output error;
the router/top-2/combine weights are exact (host fp64).
"""

import sys

sys.path.insert(0, "/opt/trn_rl_repo")

import numpy as np
import ml_dtypes

BF16 = ml_dtypes.bfloat16

# Model dims (fixed for this problem)
B = 1024          # tokens
DIN = 3072        # input features
RHID = 128        # router hidden
E = 8             # experts = cores
EHID = 2048       # expert hidden
NCLS = 10         # classes
KC1 = DIN // 128  # 24 K-chunks for DIN contraction
KC2 = EHID // 128 # 16 hid tiles per expert
HK = KC2 // 2     # 8 hid tiles per half-slot
TOPK = 2

_PROGRAMS = {}
LAST_RESULTS = None


def _ensure_axon_profile_hook():
    """bass_utils' trace=True path imports antenv.axon_hooks, which this
    image lacks. Provide it (backed by libaxon_pjrt.so's NRT profile C API)
    so NTFF profiling works; degrade silently if unavailable."""
    import contextlib
    import ctypes
    import os
    import types

    try:
        from antenv.axon_hooks import get_axon_ntff_profile_hook  # noqa: F401
        return
    except ImportError:
        pass
    try:
        import antenv
    except ImportError:
        return

    state = {"hook": None}
    mod = types.ModuleType("antenv.axon_hooks")
    mod.set_axon_ntff_profile_hook = lambda h: state.__setitem__("hook", h)
    mod.get_axon_ntff_profile_hook = lambda: state["hook"]
    sys.modules["antenv.axon_hooks"] = mod
    antenv.axon_hooks = mod

    so_path = "/opt/axon/libaxon_pjrt.so"
    if not os.path.exists(so_path):
        return
    try:
        lib = ctypes.CDLL(so_path)
    except OSError:
        return
    if not hasattr(lib, "axon_start_nrt_profile"):
        return
    lib.axon_start_nrt_profile.argtypes = [
        ctypes.POINTER(ctypes.c_int64), ctypes.c_size_t]
    lib.axon_start_nrt_profile.restype = ctypes.c_int64
    lib.axon_stop_nrt_profile.argtypes = [ctypes.c_char_p]
    lib.axon_stop_nrt_profile.restype = ctypes.c_int64

    @contextlib.contextmanager
    def _hook(output_dir, device_ids):
        import jax

        jax.devices()
        if device_ids:
            ids = (ctypes.c_int64 * len(device_ids))(*device_ids)
            rc = lib.axon_start_nrt_profile(ids, len(device_ids))
        else:
            rc = lib.axon_start_nrt_profile(None, 0)
        if rc != 0:
            raise RuntimeError(f"axon_start_nrt_profile rc={rc}")
        try:
            yield
        finally:
            n = lib.axon_stop_nrt_profile(str(output_dir).encode())
            print(f"profile: {n} ntff file(s) -> {output_dir}",
                  file=sys.stderr)

    state["hook"] = _hook


def _chunks(c):
    """Split c columns into <=512 PSUM-bank-sized pieces (each >=128 wide
    when possible so matmuls stay moving-bound)."""
    n = -(-c // 512)
    base = c // n
    out = []
    off = 0
    for i in range(n):
        w = base + (1 if i < c % n else 0)
        out.append((off, off + w))
        off += w
    return out


def _build_program(C0, C1):
    import concourse.tile as tile
    from concourse import bacc, mybir

    f32 = mybir.dt.float32
    bf = mybir.dt.bfloat16
    AF = mybir.ActivationFunctionType
    ALU = mybir.AluOpType

    NX = C0 + C1                      # x columns per core
    caps = [C0] * HK + [C1] * HK      # columns per hid tile
    offs = [0] * HK + [C0] * HK       # x column offset per hid tile
    pads = [-(-C0 // 128) * 128, -(-C1 // 128) * 128]
    ntt = pads[0] // 128 + pads[1] // 128   # total 128-token out tiles

    nc = bacc.Bacc("TRN2", debug=False, num_devices=E)

    # ---- DRAM I/O (all per-core data) ---------------------------------
    # x: [i, k, n] -- cols [0:C0] slot0 expert tokens, [C0:NX] slot1
    d_x = nc.dram_tensor("x", [128, KC1, NX], bf, kind="ExternalInput")
    # stationary tiles: t<8 slot0 expert half, t>=8 slot1 expert half
    # layout [t, i, (k j)]: element = eW1[e][128k + i, 128*m(t) + j]
    d_ew1 = nc.dram_tensor("ew1", [KC2, 128, DIN], bf, kind="ExternalInput")
    d_eb1 = nc.dram_tensor("eb1", [128, KC2], f32, kind="ExternalInput")
    # ew2[:, t, :] = eW2[e][128*m(t):128*(m(t)+1), :]
    d_ew2 = nc.dram_tensor("ew2", [128, KC2, NCLS], bf, kind="ExternalInput")
    # eb2 tiled to 128 partitions per slot
    d_eb2 = nc.dram_tensor("eb2", [128, 2, NCLS], f32, kind="ExternalInput")
    # per-token normalized routing weight, [token-in-tile, out-tile]
    d_w = nc.dram_tensor("w", [128, ntt], f32, kind="ExternalInput")
    # weighted partial rows; host scatter-adds real rows into [B, NCLS]
    d_out = nc.dram_tensor("out", [ntt * 128, NCLS], f32,
                           kind="ExternalOutput")

    with tile.TileContext(nc) as tc:
        with (
            tc.tile_pool(name="const", bufs=1) as cp,
            tc.tile_pool(name="wstream", bufs=6) as wp,
            tc.tile_pool(name="psum", bufs=1, space="PSUM") as pp,
            tc.tile_pool(name="outp", bufs=1) as op,
        ):
            # ---- HAM pre-warm: full-array K=128 dummies flip the PE
            # clock gate to 2.4 GHz during the DMA ramp (small-K matmuls
            # don't count as PE-busy).
            warmt = cp.tile([128, 128], bf, tag="warmt", name="warmt")
            nc.vector.memset(warmt[:], 1.0)
            warm = pp.tile([128, 128], f32, tag="po", bufs=2, name="warm")
            for _i in range(44):
                nc.tensor.matmul(warm[:], warmt[:], warmt[:],
                                 start=True, stop=True)

            # eh tiles, padded to 128-multiples; zero the pad columns so
            # mm2 stationary slices never read uninitialized SBUF.
            ehs = []
            for t in range(KC2):
                g = t // HK
                eh = cp.tile([128, pads[g]], bf, tag=f"eh{t}", name=f"eh{t}")
                ehs.append(eh)
                cap = caps[t]
                if cap < pads[g]:
                    nc.vector.memset(eh[:, cap:], 0.0)

            # ---- input DMA (emission order ~= DMA queue order) --------
            # mm1 t=0/1 run k-outer and consume x chunks as they arrive;
            # interleave the first ew1 tiles between x chunks.
            wts = {}

            def load_ew1(t):
                wt = wp.tile([128, DIN], bf, tag="ew1", name=f"ew1t{t}")
                nc.sync.dma_start(wt[:, :DIN // 2], d_ew1[t][:, :DIN // 2])
                nc.sync.dma_start(wt[:, DIN // 2:], d_ew1[t][:, DIN // 2:])
                wts[t] = wt

            xk = []
            for k in range(KC1):
                t = cp.tile([128, NX], bf, tag=f"xk{k}", name=f"xk{k}")
                xk.append(t)
            load_ew1(0)
            nc.sync.dma_start(xk[0][:], d_x[:, 0, :])
            nc.sync.dma_start(xk[1][:], d_x[:, 1, :])
            load_ew1(1)
            for k in range(2, KC1):
                nc.sync.dma_start(xk[k][:], d_x[:, k, :])
                if k == 4:
                    load_ew1(2)
                if k == 8:
                    load_ew1(3)
                if k == 12:
                    load_ew1(4)
                if k == 16:
                    load_ew1(5)
            eb1t = cp.tile([128, KC2], f32, tag="eb1", name="eb1t")
            nc.sync.dma_start(eb1t[:], d_eb1[:])
            ew2t = cp.tile([128, KC2, NCLS], bf, tag="ew2", name="ew2t")
            nc.sync.dma_start(ew2t[:], d_ew2[:])
            eb2t = cp.tile([128, 2, NCLS], f32, tag="eb2", name="eb2t")
            nc.sync.dma_start(eb2t[:], d_eb2[:])
            wtok = cp.tile([128, ntt], f32, tag="wtok", name="wtok")
            nc.sync.dma_start(wtok[:], d_w[:])

            def emit_mm2(g):
                # out[tok, cls] for slot g: psum rows = tokens, stationary
                # = 128-token slices of this slot's eh tiles.
                base_tt = 0 if g == 0 else pads[0] // 128
                for tt in range(pads[g] // 128):
                    po = pp.tile([128, NCLS], f32, tag="po", bufs=2,
                                 name=f"po{g}_{tt}")
                    for j in range(HK):
                        t = g * HK + j
                        nc.tensor.matmul(
                            po[:],
                            ehs[t][:, tt * 128:(tt + 1) * 128],
                            ew2t[:, t, :],
                            start=(j == 0),
                            stop=(j == HK - 1),
                        )
                    osb = op.tile([128, NCLS], f32, tag="osb", bufs=4,
                                  name=f"osb{g}_{tt}")
                    nc.vector.tensor_add(osb[:], po[:], eb2t[:, g, :])
                    col = base_tt + tt
                    nc.vector.tensor_scalar(
                        osb[:], osb[:], wtok[:, col:col + 1], None, ALU.mult)
                    nc.sync.dma_start(
                        d_out[(base_tt + tt) * 128:(base_tt + tt + 1) * 128,
                              :], osb[:])

            # ---- mm1: eh[t] = relu(W1_tile[t].T @ x_cols + b) ----------
            for t in range(KC2):
                wt = wts[t]
                if t + 6 < KC2:
                    load_ew1(t + 6)
                off, cap, g = offs[t], caps[t], t // HK
                ch = _chunks(cap)
                pss = [pp.tile([128, b - a], f32, tag="mm1", bufs=4,
                               name=f"ps1_{t}_{i}") for i, (a, b) in
                       enumerate(ch)]
                if t < 2:
                    # k-outer during the DMA ramp: consumes each x chunk
                    # at the rate it arrives
                    for k in range(KC1):
                        for i, (a, b) in enumerate(ch):
                            nc.tensor.matmul(
                                pss[i][:],
                                wt[:, k * 128:(k + 1) * 128],
                                xk[k][:, off + a:off + b],
                                start=(k == 0),
                                stop=(k == KC1 - 1),
                            )
                else:
                    for i, (a, b) in enumerate(ch):
                        for k in range(KC1):
                            nc.tensor.matmul(
                                pss[i][:],
                                wt[:, k * 128:(k + 1) * 128],
                                xk[k][:, off + a:off + b],
                                start=(k == 0),
                                stop=(k == KC1 - 1),
                            )
                for i, (a, b) in enumerate(ch):
                    nc.scalar.activation(
                        ehs[t][:, a:b], pss[i][:],
                        AF.Relu, bias=eb1t[:, t:t + 1],
                    )
                if t == HK + 1:
                    # slot0's eh tiles are complete; its mm2 hides under
                    # the remaining slot1 mm1 work
                    emit_mm2(0)
            emit_mm2(1)

    return nc


def _route_host(x, rW1, rb1, rW2, rb2):
    """Exact router on host: top-2 expert ids + normalized weights."""
    xf = np.asarray(x, np.float64).reshape(B, DIN)
    rh = np.maximum(xf @ np.asarray(rW1, np.float64)
                    + np.asarray(rb1, np.float64), 0.0)
    logits = rh @ np.asarray(rW2, np.float64) + np.asarray(rb2, np.float64)
    m = logits.max(-1, keepdims=True)
    p = np.exp(logits - m)
    p /= p.sum(-1, keepdims=True)
    idx = np.argsort(-p, axis=-1, kind="stable")[:, :TOPK]
    w = np.take_along_axis(p, idx, axis=-1)
    w /= w.sum(-1, keepdims=True)
    return idx, w


def _plan(counts):
    """Pack the 16 expert-halves into 8 (slot0, slot1) core pairs.

    Returns (C0, C1, plan) with plan[c] = [expert_slot0, expert_slot1];
    core c takes half c%2 of each. Slot0 = the 4 largest experts, slot1 =
    the 4 smallest: provably minimal C0 + C1 for this slot structure.
    """
    order = np.argsort(-counts, kind="stable")
    C0 = int(counts[order[0]])
    C1 = max(int(counts[order[4]]), 1)
    plan = []
    for c in range(E):
        plan.append((int(order[c // 2]), int(order[4 + c // 2])))
    return C0, C1, plan


def _prep_inputs(x, rW1, rb1, rW2, rb2, eW1, eb1, eW2, eb2):
    """Host routing + shard/layout prep. Returns (in_maps, scatter, ntt)."""
    idx, w = _route_host(x, rW1, rb1, rW2, rb2)
    tok_of = [np.where((idx == e).any(-1))[0] for e in range(E)]
    w_of = [w[(t := tok_of[e]), (idx[t] == e).argmax(-1)]
            for e in range(E)]
    counts = np.array([len(t) for t in tok_of])
    C0, C1, plan = _plan(counts)

    xf = np.ascontiguousarray(np.asarray(x, np.float32).reshape(B, DIN))
    # [i, k, n] layout: xt[:, k, t] = xf[t, 128k + i]
    xt = xf.reshape(B, KC1, 128).transpose(2, 1, 0).astype(BF16)

    ew1_full = [np.asarray(eW1[e], np.float32)
                .reshape(KC1, 128, KC2, 128)
                .transpose(2, 1, 0, 3)
                .reshape(KC2, 128, DIN)
                .astype(BF16) for e in range(E)]
    eb1_full = [np.asarray(eb1[e], np.float32).reshape(KC2, 128).T
                for e in range(E)]
    ew2_full = [np.asarray(eW2[e], np.float32)
                .reshape(KC2, 128, NCLS)
                .transpose(1, 0, 2)
                .astype(BF16) for e in range(E)]

    pads = [-(-C0 // 128) * 128, -(-C1 // 128) * 128]
    ntt = (pads[0] + pads[1]) // 128
    caps = [C0, C1]

    in_maps = []
    scatter = []   # per core: (row_offset, token_ids) per slot
    for c in range(E):
        xblob = np.zeros((128, KC1, C0 + C1), BF16)
        ew1b = np.empty((KC2, 128, DIN), BF16)
        eb1b = np.empty((128, KC2), np.float32)
        ew2b = np.empty((128, KC2, NCLS), BF16)
        eb2b = np.empty((128, 2, NCLS), np.float32)
        wb = np.zeros((128, ntt), np.float32)
        half = c % 2
        sc = []
        for g in range(2):
            e = plan[c][g]
            toks = tok_of[e]
            n = len(toks)
            off = 0 if g == 0 else C0
            xblob[:, :, off:off + n] = xt[:, :, toks]
            mlo = half * HK
            ew1b[g * HK:(g + 1) * HK] = ew1_full[e][mlo:mlo + HK]
            eb1b[:, g * HK:(g + 1) * HK] = eb1_full[e][:, mlo:mlo + HK]
            ew2b[:, g * HK:(g + 1) * HK] = ew2_full[e][:, mlo:mlo + HK]
            eb2b[:, g, :] = np.asarray(eb2[e], np.float32)[None, :]
            base_tt = 0 if g == 0 else pads[0] // 128
            wcol = np.zeros(pads[g], np.float32)
            wcol[:n] = w_of[e]
            wb[:, base_tt:base_tt + pads[g] // 128] = (
                wcol.reshape(-1, 128).T)
            sc.append((base_tt * 128, toks))
            assert n <= caps[g]
        in_maps.append({
            "x": np.ascontiguousarray(xblob),
            "ew1": np.ascontiguousarray(ew1b),
            "eb1": eb1b, "ew2": np.ascontiguousarray(ew2b),
            "eb2": eb2b, "w": wb,
        })
        scatter.append(sc)
    return in_maps, scatter, C0, C1


def kernel(x, rW1, rb1, rW2, rb2, eW1, eb1, eW2, eb2):
    global LAST_RESULTS
    _ensure_axon_profile_hook()
    from concourse.bass_utils import run_bass_kernel_spmd

    in_maps, scatter, C0, C1 = _prep_inputs(
        x, rW1, rb1, rW2, rb2, eW1, eb1, eW2, eb2)
    key = (C0, C1)
    nc = _PROGRAMS.get(key)
    if nc is None:
        nc = _build_program(C0, C1)
        nc.finalize()
        _PROGRAMS[key] = nc
    res = run_bass_kernel_spmd(nc, in_maps, core_ids=list(range(E)))
    LAST_RESULTS = res
    out = np.zeros((B, NCLS), np.float32)
    for c, r in enumerate(res.results):
        part = np.asarray(r["out"], np.float32)
        for (row0, toks) in scatter[c]:
            np.add.at(out, toks, part[row0:row0 + len(toks)])
    return out


# revision 10
# speedup vs baseline: 2.6890x; 1.1370x over previous
"""Trainium2 Bass kernel for the MoE routing module (nn_MoE_53042846105633).

Strategy: top-2 sparse dispatch, expert-parallel across 8 NeuronCores.
The router (0.8% of reference FLOPs) runs on host in fp64 during input
sharding -- the sharding_hint's "dispatch of tokens by top-k expert id"
-- which cuts device matmul work 4x vs dense: only the 2048 routed
(token, expert) pairs are computed instead of 8 experts x 1024 tokens.

Token counts per expert are skewed (e.g. [199,263,548,380,142,99,271,146]
for the seed-0 data), so each expert's 16 hidden tiles are split into two
8-tile halves and the 16 halves are packed onto cores as (big, small)
slot pairs: slot0 capacity C0 = largest expert count, slot1 capacity C1 =
5th-largest. All 8 cores run the identical instruction stream (SPMD);
per-core data (which expert half + which token columns) differs. With 16
half-slots and >=2 slots per expert this (C0 + C1) packing is optimal.

Per core: mm1 = 8 hid-tiles x 24 K-chunks over C0 cols + 8 x 24 over C1
(~60us PE at bf16), relu -> eh bf16, mm2 contracts eh against eW2 per
128-token stationary slice, bias + per-token routing weight (0 on padding)
-> weighted partial rows DMA'd out; host scatter-adds them into [B, 10].
Weights stream from HBM (12.6MB/core) well under the mm1 time. No
collectives (a NEFF with collectives drops the PE to 2.0 GHz). Expert
math is plain bf16 (fp32 PSUM accumulate), ~3e-3 relative output error;
the router/top-2/combine weights are exact (host fp64).
"""

import sys

sys.path.insert(0, "/opt/trn_rl_repo")

import numpy as np
import ml_dtypes

BF16 = ml_dtypes.bfloat16

# Model dims (fixed for this problem)
B = 1024          # tokens
DIN = 3072        # input features
RHID = 128        # router hidden
E = 8             # experts = cores
EHID = 2048       # expert hidden
NCLS = 10         # classes
KC1 = DIN // 128  # 24 K-chunks for DIN contraction
KC2 = EHID // 128 # 16 hid tiles per expert
HK = KC2 // 2     # 8 hid tiles per half-slot
TOPK = 2

_PROGRAMS = {}
LAST_RESULTS = None


def _ensure_axon_profile_hook():
    """bass_utils' trace=True path imports antenv.axon_hooks, which this
    image lacks. Provide it (backed by libaxon_pjrt.so's NRT profile C API)
    so NTFF profiling works; degrade silently if unavailable."""
    import contextlib
    import ctypes
    import os
    import types

    try:
        from antenv.axon_hooks import get_axon_ntff_profile_hook  # noqa: F401
        return
    except ImportError:
        pass
    try:
        import antenv
    except ImportError:
        return

    state = {"hook": None}
    mod = types.ModuleType("antenv.axon_hooks")
    mod.set_axon_ntff_profile_hook = lambda h: state.__setitem__("hook", h)
    mod.get_axon_ntff_profile_hook = lambda: state["hook"]
    sys.modules["antenv.axon_hooks"] = mod
    antenv.axon_hooks = mod

    so_path = "/opt/axon/libaxon_pjrt.so"
    if not os.path.exists(so_path):
        return
    try:
        lib = ctypes.CDLL(so_path)
    except OSError:
        return
    if not hasattr(lib, "axon_start_nrt_profile"):
        return
    lib.axon_start_nrt_profile.argtypes = [
        ctypes.POINTER(ctypes.c_int64), ctypes.c_size_t]
    lib.axon_start_nrt_profile.restype = ctypes.c_int64
    lib.axon_stop_nrt_profile.argtypes = [ctypes.c_char_p]
    lib.axon_stop_nrt_profile.restype = ctypes.c_int64

    @contextlib.contextmanager
    def _hook(output_dir, device_ids):
        import jax

        jax.devices()
        if device_ids:
            ids = (ctypes.c_int64 * len(device_ids))(*device_ids)
            rc = lib.axon_start_nrt_profile(ids, len(device_ids))
        else:
            rc = lib.axon_start_nrt_profile(None, 0)
        if rc != 0:
            raise RuntimeError(f"axon_start_nrt_profile rc={rc}")
        try:
            yield
        finally:
            n = lib.axon_stop_nrt_profile(str(output_dir).encode())
            print(f"profile: {n} ntff file(s) -> {output_dir}",
                  file=sys.stderr)

    state["hook"] = _hook


def _chunks(c):
    """Split c columns into <=512 PSUM-bank-sized pieces (each >=128 wide
    when possible so matmuls stay moving-bound)."""
    n = -(-c // 512)
    base = c // n
    out = []
    off = 0
    for i in range(n):
        w = base + (1 if i < c % n else 0)
        out.append((off, off + w))
        off += w
    return out


def _build_program(C0, C1):
    import concourse.tile as tile
    from concourse import bacc, mybir

    f32 = mybir.dt.float32
    bf = mybir.dt.bfloat16
    AF = mybir.ActivationFunctionType
    ALU = mybir.AluOpType

    caps = [C0] * HK + [C1] * HK      # columns per hid tile
    pads = [-(-C0 // 128) * 128, -(-C1 // 128) * 128]
    ntt = pads[0] // 128 + pads[1] // 128   # total 128-token out tiles
    RAMP = 4                          # slot0 tiles run k-outer in the ramp

    nc = bacc.Bacc("TRN2", debug=False, num_devices=E)

    # ---- DRAM I/O (all per-core data) ---------------------------------
    # x: [i, k, n]; slot0 tokens streamed per-k (paces the ramp k-outer),
    # slot1 tokens as one late transfer (first needed ~halfway through)
    d_x0 = nc.dram_tensor("x0", [128, KC1, C0], bf, kind="ExternalInput")
    d_x1 = nc.dram_tensor("x1", [128, KC1, C1], bf, kind="ExternalInput")
    # stationary tiles: t<8 slot0 expert half, t>=8 slot1 expert half
    # layout [t, i, (k j)]: element = eW1[e][128k + i, 128*m(t) + j]
    d_ew1 = nc.dram_tensor("ew1", [KC2, 128, DIN], bf, kind="ExternalInput")
    d_eb1 = nc.dram_tensor("eb1", [128, KC2], f32, kind="ExternalInput")
    # ew2[:, t, :] = eW2[e][128*m(t):128*(m(t)+1), :]
    d_ew2 = nc.dram_tensor("ew2", [128, KC2, NCLS], bf, kind="ExternalInput")
    # eb2 tiled to 128 partitions per slot
    d_eb2 = nc.dram_tensor("eb2", [128, 2, NCLS], f32, kind="ExternalInput")
    # per-token normalized routing weight, [token-in-tile, out-tile]
    d_w = nc.dram_tensor("w", [128, ntt], f32, kind="ExternalInput")
    # weighted partial rows; host scatter-adds real rows into [B, NCLS]
    d_out = nc.dram_tensor("out", [ntt * 128, NCLS], f32,
                           kind="ExternalOutput")

    with tile.TileContext(nc) as tc:
        with (
            tc.tile_pool(name="const", bufs=1) as cp,
            tc.tile_pool(name="wstream", bufs=6) as wp,
            tc.tile_pool(name="psum", bufs=1, space="PSUM") as pp,
            tc.tile_pool(name="outp", bufs=1) as op,
        ):
            # ---- HAM pre-warm: full-array K=128 dummies flip the PE
            # clock gate to 2.4 GHz during the DMA ramp (small-K matmuls
            # don't count as PE-busy).
            warmt = cp.tile([128, 128], bf, tag="warmt", name="warmt")
            nc.vector.memset(warmt[:], 1.0)
            warm = pp.tile([128, 128], f32, tag="mm1", bufs=8, name="warm")
            for _i in range(44):
                nc.tensor.matmul(warm[:], warmt[:], warmt[:],
                                 start=True, stop=True)

            # eh tiles, padded to 128-multiples; zero the pad columns so
            # mm2 stationary slices never read uninitialized SBUF.
            ehs = []
            for t in range(KC2):
                g = t // HK
                eh = cp.tile([128, pads[g]], bf, tag=f"eh{t}", name=f"eh{t}")
                ehs.append(eh)
                cap = caps[t]
                if cap < pads[g]:
                    nc.vector.memset(eh[:, cap:], 0.0)

            # ---- input DMA (emission order = FIFO queue order) --------
            # All 16 ew1 tiles are SBUF-resident (no WAR queue stalls);
            # slot0 x chunks pace the ramp k-outer; everything slot1
            # needs comes after the slot0-critical stream.
            wts = {}

            def load_ew1(t):
                wt = wp.tile([128, DIN], bf, tag="ew1", name=f"ew1t{t}")
                nc.sync.dma_start(wt[:, :DIN // 2], d_ew1[t][:, :DIN // 2])
                nc.sync.dma_start(wt[:, DIN // 2:], d_ew1[t][:, DIN // 2:])
                wts[t] = wt

            eb1t = cp.tile([128, KC2], f32, tag="eb1", name="eb1t")
            nc.sync.dma_start(eb1t[:], d_eb1[:])
            x0k = []
            for k in range(KC1):
                t = cp.tile([128, C0], bf, tag=f"x0k{k}", name=f"x0k{k}")
                x0k.append(t)
            load_ew1(0)
            nc.sync.dma_start(x0k[0][:], d_x0[:, 0, :])
            load_ew1(1)
            nc.sync.dma_start(x0k[1][:], d_x0[:, 1, :])
            nc.sync.dma_start(x0k[2][:], d_x0[:, 2, :])
            load_ew1(2)
            for k in range(3, 6):
                nc.sync.dma_start(x0k[k][:], d_x0[:, k, :])
            load_ew1(3)
            for k in range(6, 11):
                nc.sync.dma_start(x0k[k][:], d_x0[:, k, :])
            load_ew1(4)
            for k in range(11, 17):
                nc.sync.dma_start(x0k[k][:], d_x0[:, k, :])
            load_ew1(5)
            for k in range(17, KC1):
                nc.sync.dma_start(x0k[k][:], d_x0[:, k, :])
            load_ew1(6)
            load_ew1(7)
            ew2t = cp.tile([128, KC2, NCLS], bf, tag="ew2", name="ew2t")
            nc.sync.dma_start(ew2t[:], d_ew2[:])
            eb2t = cp.tile([128, 2, NCLS], f32, tag="eb2", name="eb2t")
            nc.sync.dma_start(eb2t[:], d_eb2[:])
            wtok = cp.tile([128, ntt], f32, tag="wtok", name="wtok")
            nc.sync.dma_start(wtok[:], d_w[:])
            load_ew1(8)
            x1t = cp.tile([128, KC1, C1], bf, tag="x1t", name="x1t")
            nc.sync.dma_start(x1t[:], d_x1[:])
            for t in range(9, KC2):
                load_ew1(t)

            def emit_mm2(g):
                # out[tok, cls] for slot g: psum rows = tokens, stationary
                # = 128-token slices of this slot's eh tiles.
                base_tt = 0 if g == 0 else pads[0] // 128
                for tt in range(pads[g] // 128):
                    po = pp.tile([128, NCLS], f32, tag="mm1", bufs=8,
                                 name=f"po{g}_{tt}")
                    for j in range(HK):
                        t = g * HK + j
                        nc.tensor.matmul(
                            po[:],
                            ehs[t][:, tt * 128:(tt + 1) * 128],
                            ew2t[:, t, :],
                            start=(j == 0),
                            stop=(j == HK - 1),
                        )
                    osb = op.tile([128, NCLS], f32, tag="osb", bufs=4,
                                  name=f"osb{g}_{tt}")
                    nc.vector.tensor_add(osb[:], po[:], eb2t[:, g, :])
                    col = base_tt + tt
                    nc.vector.tensor_scalar(
                        osb[:], osb[:], wtok[:, col:col + 1], None, ALU.mult)
                    nc.sync.dma_start(
                        d_out[(base_tt + tt) * 128:(base_tt + tt + 1) * 128,
                              :], osb[:])

            # ---- mm1: eh[t] = relu(W1_tile[t].T @ x_cols + b) ----------
            def xsrc(t, k, a, b):
                return (x0k[k][:, a:b] if t < HK
                        else x1t[:, k, a:b])

            ch0 = _chunks(C0)
            # ramp: first RAMP slot0 tiles jointly k-outer, consuming
            # each x chunk as it arrives (PE pace >= DMA supply)
            ramp_pss = {t: [pp.tile([128, b - a], f32, tag="mm1", bufs=8,
                                    name=f"ps1_{t}_{i}")
                            for i, (a, b) in enumerate(ch0)]
                        for t in range(RAMP)}
            for k in range(KC1):
                for t in range(RAMP):
                    for i, (a, b) in enumerate(ch0):
                        nc.tensor.matmul(
                            ramp_pss[t][i][:],
                            wts[t][:, k * 128:(k + 1) * 128],
                            x0k[k][:, a:b],
                            start=(k == 0),
                            stop=(k == KC1 - 1),
                        )
            for t in range(RAMP):
                for i, (a, b) in enumerate(ch0):
                    nc.scalar.activation(
                        ehs[t][:, a:b], ramp_pss[t][i][:],
                        AF.Relu, bias=eb1t[:, t:t + 1],
                    )

            for t in range(RAMP, KC2):
                wt = wts[t]
                cap = caps[t]
                ch = _chunks(cap)
                pss = [pp.tile([128, b - a], f32, tag="mm1", bufs=8,
                               name=f"ps1_{t}_{i}") for i, (a, b) in
                       enumerate(ch)]
                for i, (a, b) in enumerate(ch):
                    for k in range(KC1):
                        nc.tensor.matmul(
                            pss[i][:],
                            wt[:, k * 128:(k + 1) * 128],
                            xsrc(t, k, a, b),
                            start=(k == 0),
                            stop=(k == KC1 - 1),
                        )
                for i, (a, b) in enumerate(ch):
                    nc.scalar.activation(
                        ehs[t][:, a:b], pss[i][:],
                        AF.Relu, bias=eb1t[:, t:t + 1],
                    )
                if t == HK + 1:
                    # slot0's eh tiles are complete; its mm2 hides under
                    # the remaining slot1 mm1 work
                    emit_mm2(0)
            emit_mm2(1)

    return nc


def _route_host(x, rW1, rb1, rW2, rb2):
    """Exact router on host: top-2 expert ids + normalized weights."""
    xf = np.asarray(x, np.float64).reshape(B, DIN)
    rh = np.maximum(xf @ np.asarray(rW1, np.float64)
                    + np.asarray(rb1, np.float64), 0.0)
    logits = rh @ np.asarray(rW2, np.float64) + np.asarray(rb2, np.float64)
    m = logits.max(-1, keepdims=True)
    p = np.exp(logits - m)
    p /= p.sum(-1, keepdims=True)
    idx = np.argsort(-p, axis=-1, kind="stable")[:, :TOPK]
    w = np.take_along_axis(p, idx, axis=-1)
    w /= w.sum(-1, keepdims=True)
    return idx, w


def _plan(counts):
    """Pack the 16 expert-halves into 8 (slot0, slot1) core pairs.

    Returns (C0, C1, plan) with plan[c] = [expert_slot0, expert_slot1];
    core c takes half c%2 of each. Slot0 = the 4 largest experts, slot1 =
    the 4 smallest: provably minimal C0 + C1 for this slot structure.
    """
    order = np.argsort(-counts, kind="stable")
    C0 = int(counts[order[0]])
    C1 = max(int(counts[order[4]]), 1)
    plan = []
    for c in range(E):
        plan.append((int(order[c // 2]), int(order[4 + c // 2])))
    return C0, C1, plan


def _prep_inputs(x, rW1, rb1, rW2, rb2, eW1, eb1, eW2, eb2):
    """Host routing + shard/layout prep. Returns (in_maps, scatter, ntt)."""
    idx, w = _route_host(x, rW1, rb1, rW2, rb2)
    tok_of = [np.where((idx == e).any(-1))[0] for e in range(E)]
    w_of = [w[(t := tok_of[e]), (idx[t] == e).argmax(-1)]
            for e in range(E)]
    counts = np.array([len(t) for t in tok_of])
    C0, C1, plan = _plan(counts)

    xf = np.ascontiguousarray(np.asarray(x, np.float32).reshape(B, DIN))
    # [i, k, n] layout: xt[:, k, t] = xf[t, 128k + i]
    xt = xf.reshape(B, KC1, 128).transpose(2, 1, 0).astype(BF16)

    ew1_full = [np.asarray(eW1[e], np.float32)
                .reshape(KC1, 128, KC2, 128)
                .transpose(2, 1, 0, 3)
                .reshape(KC2, 128, DIN)
                .astype(BF16) for e in range(E)]
    eb1_full = [np.asarray(eb1[e], np.float32).reshape(KC2, 128).T
                for e in range(E)]
    ew2_full = [np.asarray(eW2[e], np.float32)
                .reshape(KC2, 128, NCLS)
                .transpose(1, 0, 2)
                .astype(BF16) for e in range(E)]

    pads = [-(-C0 // 128) * 128, -(-C1 // 128) * 128]
    ntt = (pads[0] + pads[1]) // 128
    caps = [C0, C1]

    in_maps = []
    scatter = []   # per core: (row_offset, token_ids) per slot
    for c in range(E):
        xblob = [np.zeros((128, KC1, C0), BF16),
                 np.zeros((128, KC1, C1), BF16)]
        ew1b = np.empty((KC2, 128, DIN), BF16)
        eb1b = np.empty((128, KC2), np.float32)
        ew2b = np.empty((128, KC2, NCLS), BF16)
        eb2b = np.empty((128, 2, NCLS), np.float32)
        wb = np.zeros((128, ntt), np.float32)
        half = c % 2
        sc = []
        for g in range(2):
            e = plan[c][g]
            toks = tok_of[e]
            n = len(toks)
            xblob[g][:, :, :n] = xt[:, :, toks]
            mlo = half * HK
            ew1b[g * HK:(g + 1) * HK] = ew1_full[e][mlo:mlo + HK]
            eb1b[:, g * HK:(g + 1) * HK] = eb1_full[e][:, mlo:mlo + HK]
            ew2b[:, g * HK:(g + 1) * HK] = ew2_full[e][:, mlo:mlo + HK]
            eb2b[:, g, :] = np.asarray(eb2[e], np.float32)[None, :]
            base_tt = 0 if g == 0 else pads[0] // 128
            wcol = np.zeros(pads[g], np.float32)
            wcol[:n] = w_of[e]
            wb[:, base_tt:base_tt + pads[g] // 128] = (
                wcol.reshape(-1, 128).T)
            sc.append((base_tt * 128, toks))
            assert n <= caps[g]
        in_maps.append({
            "x0": np.ascontiguousarray(xblob[0]),
            "x1": np.ascontiguousarray(xblob[1]),
            "ew1": np.ascontiguousarray(ew1b),
            "eb1": eb1b, "ew2": np.ascontiguousarray(ew2b),
            "eb2": eb2b, "w": wb,
        })
        scatter.append(sc)
    return in_maps, scatter, C0, C1


def kernel(x, rW1, rb1, rW2, rb2, eW1, eb1, eW2, eb2):
    global LAST_RESULTS
    _ensure_axon_profile_hook()
    from concourse.bass_utils import run_bass_kernel_spmd

    in_maps, scatter, C0, C1 = _prep_inputs(
        x, rW1, rb1, rW2, rb2, eW1, eb1, eW2, eb2)
    key = (C0, C1)
    nc = _PROGRAMS.get(key)
    if nc is None:
        nc = _build_program(C0, C1)
        nc.finalize()
        _PROGRAMS[key] = nc
    res = run_bass_kernel_spmd(nc, in_maps, core_ids=list(range(E)))
    LAST_RESULTS = res
    out = np.zeros((B, NCLS), np.float32)
    for c, r in enumerate(res.results):
        part = np.asarray(r["out"], np.float32)
        for (row0, toks) in scatter[c]:
            np.add.at(out, toks, part[row0:row0 + len(toks)])
    return out


# revision 18
# speedup vs baseline: 2.7027x; 1.0051x over previous
"""Trainium2 Bass kernel for the MoE routing module (nn_MoE_53042846105633).

Strategy: top-2 sparse dispatch, expert-parallel across 8 NeuronCores.
The router (0.8% of reference FLOPs) runs on host in fp64 during input
sharding -- the sharding_hint's "dispatch of tokens by top-k expert id"
-- which cuts device matmul work 4x vs dense: only the 2048 routed
(token, expert) pairs are computed instead of 8 experts x 1024 tokens.

Token counts per expert are skewed (e.g. [199,263,548,380,142,99,271,146]
for the seed-0 data), so each expert's 16 hidden tiles are split into two
8-tile halves and the 16 halves are packed onto cores as (big, small)
slot pairs: slot0 capacity C0 = largest expert count, slot1 capacity C1 =
5th-largest. All 8 cores run the identical instruction stream (SPMD);
per-core data (which expert half + which token columns) differs. With 16
half-slots and >=2 slots per expert this (C0 + C1) packing is optimal.

Per core: mm1 = 8 hid-tiles x 24 K-chunks over C0 cols + 8 x 24 over C1
(~60us PE at bf16), relu -> eh bf16, mm2 contracts eh against eW2 per
128-token stationary slice, bias + per-token routing weight (0 on padding)
-> weighted partial rows DMA'd out; host scatter-adds them into [B, 10].
Weights stream from HBM (12.6MB/core) well under the mm1 time. No
collectives (a NEFF with collectives drops the PE to 2.0 GHz). Expert
math is plain bf16 (fp32 PSUM accumulate), ~3e-3 relative output error;
the router/top-2/combine weights are exact (host fp64).
"""

import sys

sys.path.insert(0, "/opt/trn_rl_repo")

import numpy as np
import ml_dtypes

BF16 = ml_dtypes.bfloat16

# Model dims (fixed for this problem)
B = 1024          # tokens
DIN = 3072        # input features
RHID = 128        # router hidden
E = 8             # experts = cores
EHID = 2048       # expert hidden
NCLS = 10         # classes
KC1 = DIN // 128  # 24 K-chunks for DIN contraction
KC2 = EHID // 128 # 16 hid tiles per expert
HK = KC2 // 2     # 8 hid tiles per half-slot
TOPK = 2

_PROGRAMS = {}
LAST_RESULTS = None


def _ensure_axon_profile_hook():
    """bass_utils' trace=True path imports antenv.axon_hooks, which this
    image lacks. Provide it (backed by libaxon_pjrt.so's NRT profile C API)
    so NTFF profiling works; degrade silently if unavailable."""
    import contextlib
    import ctypes
    import os
    import types

    try:
        from antenv.axon_hooks import get_axon_ntff_profile_hook  # noqa: F401
        return
    except ImportError:
        pass
    try:
        import antenv
    except ImportError:
        return

    state = {"hook": None}
    mod = types.ModuleType("antenv.axon_hooks")
    mod.set_axon_ntff_profile_hook = lambda h: state.__setitem__("hook", h)
    mod.get_axon_ntff_profile_hook = lambda: state["hook"]
    sys.modules["antenv.axon_hooks"] = mod
    antenv.axon_hooks = mod

    so_path = "/opt/axon/libaxon_pjrt.so"
    if not os.path.exists(so_path):
        return
    try:
        lib = ctypes.CDLL(so_path)
    except OSError:
        return
    if not hasattr(lib, "axon_start_nrt_profile"):
        return
    lib.axon_start_nrt_profile.argtypes = [
        ctypes.POINTER(ctypes.c_int64), ctypes.c_size_t]
    lib.axon_start_nrt_profile.restype = ctypes.c_int64
    lib.axon_stop_nrt_profile.argtypes = [ctypes.c_char_p]
    lib.axon_stop_nrt_profile.restype = ctypes.c_int64

    @contextlib.contextmanager
    def _hook(output_dir, device_ids):
        import jax

        jax.devices()
        if device_ids:
            ids = (ctypes.c_int64 * len(device_ids))(*device_ids)
            rc = lib.axon_start_nrt_profile(ids, len(device_ids))
        else:
            rc = lib.axon_start_nrt_profile(None, 0)
        if rc != 0:
            raise RuntimeError(f"axon_start_nrt_profile rc={rc}")
        try:
            yield
        finally:
            n = lib.axon_stop_nrt_profile(str(output_dir).encode())
            print(f"profile: {n} ntff file(s) -> {output_dir}",
                  file=sys.stderr)

    state["hook"] = _hook


def _chunks(c):
    """Split c columns into <=512 PSUM-bank-sized pieces (each >=128 wide
    when possible so matmuls stay moving-bound)."""
    n = -(-c // 512)
    base = c // n
    out = []
    off = 0
    for i in range(n):
        w = base + (1 if i < c % n else 0)
        out.append((off, off + w))
        off += w
    return out


def _build_program(C0, C1):
    import concourse.tile as tile
    from concourse import bacc, mybir

    f32 = mybir.dt.float32
    bf = mybir.dt.bfloat16
    AF = mybir.ActivationFunctionType
    ALU = mybir.AluOpType

    caps = [C0] * HK + [C1] * HK      # columns per hid tile
    pads = [-(-C0 // 128) * 128, -(-C1 // 128) * 128]
    ntt = pads[0] // 128 + pads[1] // 128   # total 128-token out tiles
    RAMP = 4                          # slot0 tiles run k-outer in the ramp

    nc = bacc.Bacc("TRN2", debug=False, num_devices=E)

    # ---- DRAM I/O (all per-core data) ---------------------------------
    # x: [i, k, n]; slot0 tokens streamed per-k (paces the ramp k-outer),
    # slot1 tokens as one late transfer (first needed ~halfway through)
    d_x0 = nc.dram_tensor("x0", [128, KC1, C0], bf, kind="ExternalInput")
    d_x1 = nc.dram_tensor("x1", [128, KC1, C1], bf, kind="ExternalInput")
    # stationary tiles: t<8 slot0 expert half, t>=8 slot1 expert half
    # layout [t, i, (k j)]: element = eW1[e][128k + i, 128*m(t) + j]
    d_ew1 = nc.dram_tensor("ew1", [KC2, 128, DIN], bf, kind="ExternalInput")
    d_eb1 = nc.dram_tensor("eb1", [128, KC2], f32, kind="ExternalInput")
    # ew2[:, t, :] = eW2[e][128*m(t):128*(m(t)+1), :]
    d_ew2 = nc.dram_tensor("ew2", [128, KC2, NCLS], bf, kind="ExternalInput")
    # eb2 tiled to 128 partitions per slot
    d_eb2 = nc.dram_tensor("eb2", [128, 2, NCLS], f32, kind="ExternalInput")
    # per-token normalized routing weight, [token-in-tile, out-tile]
    d_w = nc.dram_tensor("w", [128, ntt], f32, kind="ExternalInput")
    # weighted partial rows; host scatter-adds real rows into [B, NCLS]
    d_out = nc.dram_tensor("out", [ntt * 128, NCLS], f32,
                           kind="ExternalOutput")

    with tile.TileContext(nc) as tc:
        with (
            tc.tile_pool(name="const", bufs=1) as cp,
            tc.tile_pool(name="wstream", bufs=6) as wp,
            tc.tile_pool(name="psum", bufs=1, space="PSUM") as pp,
            tc.tile_pool(name="outp", bufs=1) as op,
        ):
            # ---- HAM pre-warm: full-array K=128 dummies flip the PE
            # clock gate to 2.4 GHz during the DMA ramp (small-K matmuls
            # don't count as PE-busy).
            warmt = cp.tile([128, 128], bf, tag="warmt", name="warmt")
            nc.vector.memset(warmt[:], 1.0)
            warm = pp.tile([128, 128], f32, tag="mm1", bufs=8, name="warm")
            for _i in range(58):
                nc.tensor.matmul(warm[:], warmt[:], warmt[:],
                                 start=True, stop=True)

            # eh tiles, padded to 128-multiples; zero the pad columns so
            # mm2 stationary slices never read uninitialized SBUF.
            ehs = []
            for t in range(KC2):
                g = t // HK
                eh = cp.tile([128, pads[g]], bf, tag=f"eh{t}", name=f"eh{t}")
                ehs.append(eh)
                cap = caps[t]
                if cap < pads[g]:
                    nc.vector.memset(eh[:, cap:], 0.0)

            # ---- input DMA (emission order = FIFO queue order) --------
            # All 16 ew1 tiles are SBUF-resident (no WAR queue stalls);
            # slot0 x chunks pace the ramp k-outer; everything slot1
            # needs comes after the slot0-critical stream.
            wts = {}

            def load_ew1(t):
                wt = wp.tile([128, DIN], bf, tag="ew1", name=f"ew1t{t}")
                nc.sync.dma_start(wt[:, :DIN // 2], d_ew1[t][:, :DIN // 2])
                nc.sync.dma_start(wt[:, DIN // 2:], d_ew1[t][:, DIN // 2:])
                wts[t] = wt

            # Arrival model (330 GB/s sustained, first packet ~9.3us) used
            # to order the ramp's matmul emission by data readiness.
            arr = {}
            cum = [9300.0]

            def track(key, nbytes):
                cum[0] += nbytes / 0.33e3
                arr[key] = cum[0]

            eb1t = cp.tile([128, KC2], f32, tag="eb1", name="eb1t")
            nc.sync.dma_start(eb1t[:], d_eb1[:])
            track("eb1", KC2 * 512)
            x0k = []
            for k in range(KC1):
                t = cp.tile([128, C0], bf, tag=f"x0k{k}", name=f"x0k{k}")
                x0k.append(t)

            def load_x0(k):
                nc.sync.dma_start(x0k[k][:], d_x0[:, k, :])
                track(("x0", k), 128 * C0 * 2)

            def load_ew1_tracked(t):
                load_ew1(t)
                track(("ew1", t), 128 * DIN * 2)

            load_ew1_tracked(0)
            load_ew1_tracked(1)
            for k in range(0, 5):
                load_x0(k)
            load_ew1_tracked(2)
            for k in range(5, 10):
                load_x0(k)
            load_ew1_tracked(3)
            for k in range(10, KC1):
                load_x0(k)
            for t in range(4, HK):
                load_ew1_tracked(t)
            ew2t = cp.tile([128, KC2, NCLS], bf, tag="ew2", name="ew2t")
            nc.sync.dma_start(ew2t[:], d_ew2[:])
            eb2t = cp.tile([128, 2, NCLS], f32, tag="eb2", name="eb2t")
            nc.sync.dma_start(eb2t[:], d_eb2[:])
            wtok = cp.tile([128, ntt], f32, tag="wtok", name="wtok")
            nc.sync.dma_start(wtok[:], d_w[:])
            load_ew1(8)
            x1t = cp.tile([128, KC1, C1], bf, tag="x1t", name="x1t")
            nc.sync.dma_start(x1t[:], d_x1[:])
            for t in range(9, KC2):
                load_ew1(t)

            pos_g1 = {}

            def emit_mm2_g0():
                # out[tok, cls]: psum rows = tokens, stationary = 128-token
                # slices of this slot's eh tiles.
                for tt in range(pads[0] // 128):
                    po = pp.tile([128, NCLS], f32, tag="mm1", bufs=8,
                                 name=f"po0_{tt}")
                    for j in range(HK):
                        nc.tensor.matmul(
                            po[:],
                            ehs[j][:, tt * 128:(tt + 1) * 128],
                            ew2t[:, j, :],
                            start=(j == 0),
                            stop=(j == HK - 1),
                        )
                    osb = op.tile([128, NCLS], f32, tag="osb", bufs=4,
                                  name=f"osb0_{tt}")
                    nc.vector.tensor_add(osb[:], po[:], eb2t[:, 0, :])
                    nc.vector.tensor_scalar(
                        osb[:], osb[:], wtok[:, tt:tt + 1], None, ALU.mult)
                    nc.sync.dma_start(
                        d_out[tt * 128:(tt + 1) * 128, :], osb[:])

            def emit_mm2_g1(js, finish):
                # slot1 mm2, split so only `js` of the contraction runs
                # after the last mm1 tile; one merged out DMA at the end
                # (rows interleaved as base + ntt1*token + tt; the host
                # scatter indexes accordingly).
                base = pads[0] // 128
                ntt1 = pads[1] // 128
                for tt in range(ntt1):
                    po = pos_g1.get(tt)
                    if po is None:
                        po = pp.tile([128, NCLS], f32, tag="mm1", bufs=8,
                                     name=f"po1_{tt}")
                        pos_g1[tt] = po
                    for j in js:
                        nc.tensor.matmul(
                            po[:],
                            ehs[HK + j][:, tt * 128:(tt + 1) * 128],
                            ew2t[:, HK + j, :],
                            start=(j == 0),
                            stop=(finish and j == js[-1]),
                        )
                if not finish:
                    return
                osb1 = op.tile([128, ntt1, NCLS], f32, tag="osb1",
                               name="osb1")
                for tt in range(ntt1):
                    nc.vector.tensor_add(osb1[:, tt, :], pos_g1[tt][:],
                                         eb2t[:, 1, :])
                    nc.vector.tensor_scalar(
                        osb1[:, tt, :], osb1[:, tt, :],
                        wtok[:, base + tt:base + tt + 1], None, ALU.mult)
                nc.sync.dma_start(d_out[base * 128:, :], osb1[:])

            # ---- mm1: eh[t] = relu(W1_tile[t].T @ x_cols + b) ----------
            def xsrc(t, k, a, b):
                return (x0k[k][:, a:b] if t < HK
                        else x1t[:, k, a:b])

            ch0 = _chunks(C0)
            # ramp: first RAMP slot0 tiles jointly k-outer; cells (t, k)
            # emitted in predicted-data-arrival order so the PE never
            # idles (idle gaps also drop the HAM clock below 2.4 GHz)
            ramp_pss = {t: [pp.tile([128, b - a], f32, tag="mm1", bufs=8,
                                    name=f"ps1_{t}_{i}")
                            for i, (a, b) in enumerate(ch0)]
                        for t in range(RAMP)}
            cells = sorted(
                ((max(arr[("ew1", t)], arr[("x0", k)]), k, t)
                 for t in range(RAMP) for k in range(KC1)),
                key=lambda c: (c[0], c[1], c[2]))
            seen = {t: 0 for t in range(RAMP)}
            for _, k, t in cells:
                seen[t] += 1
                for i, (a, b) in enumerate(ch0):
                    nc.tensor.matmul(
                        ramp_pss[t][i][:],
                        wts[t][:, k * 128:(k + 1) * 128],
                        x0k[k][:, a:b],
                        start=(seen[t] == 1),
                        stop=(seen[t] == KC1),
                    )
            for t in range(RAMP):
                for i, (a, b) in enumerate(ch0):
                    nc.scalar.activation(
                        ehs[t][:, a:b], ramp_pss[t][i][:],
                        AF.Relu, bias=eb1t[:, t:t + 1],
                    )

            for t in range(RAMP, KC2):
                wt = wts[t]
                cap = caps[t]
                ch = _chunks(cap)
                pss = [pp.tile([128, b - a], f32, tag="mm1", bufs=8,
                               name=f"ps1_{t}_{i}") for i, (a, b) in
                       enumerate(ch)]
                for i, (a, b) in enumerate(ch):
                    for k in range(KC1):
                        nc.tensor.matmul(
                            pss[i][:],
                            wt[:, k * 128:(k + 1) * 128],
                            xsrc(t, k, a, b),
                            start=(k == 0),
                            stop=(k == KC1 - 1),
                        )
                for i, (a, b) in enumerate(ch):
                    nc.scalar.activation(
                        ehs[t][:, a:b], pss[i][:],
                        AF.Relu, bias=eb1t[:, t:t + 1],
                    )
                if t == HK + 1:
                    # slot0's eh tiles are complete; its mm2 hides under
                    # the remaining slot1 mm1 work
                    emit_mm2_g0()
                if t == KC2 - 5:
                    # pre-accumulate the first half of slot1's mm2 so
                    # only 4 matmuls per out-tile trail the last mm1 tile
                    emit_mm2_g1(list(range(4)), finish=False)
            emit_mm2_g1(list(range(4, HK)), finish=True)

    return nc


def _route_host(x, rW1, rb1, rW2, rb2):
    """Exact router on host: top-2 expert ids + normalized weights."""
    xf = np.asarray(x, np.float64).reshape(B, DIN)
    rh = np.maximum(xf @ np.asarray(rW1, np.float64)
                    + np.asarray(rb1, np.float64), 0.0)
    logits = rh @ np.asarray(rW2, np.float64) + np.asarray(rb2, np.float64)
    m = logits.max(-1, keepdims=True)
    p = np.exp(logits - m)
    p /= p.sum(-1, keepdims=True)
    idx = np.argsort(-p, axis=-1, kind="stable")[:, :TOPK]
    w = np.take_along_axis(p, idx, axis=-1)
    w /= w.sum(-1, keepdims=True)
    return idx, w


def _plan(counts):
    """Pack the 16 expert-halves into 8 (slot0, slot1) core pairs.

    Returns (C0, C1, plan) with plan[c] = [expert_slot0, expert_slot1];
    core c takes half c%2 of each. Slot0 = the 4 largest experts, slot1 =
    the 4 smallest: provably minimal C0 + C1 for this slot structure.
    """
    order = np.argsort(-counts, kind="stable")
    C0 = int(counts[order[0]])
    C1 = max(int(counts[order[4]]), 1)
    plan = []
    for c in range(E):
        plan.append((int(order[c // 2]), int(order[4 + c // 2])))
    return C0, C1, plan


def _prep_inputs(x, rW1, rb1, rW2, rb2, eW1, eb1, eW2, eb2):
    """Host routing + shard/layout prep. Returns (in_maps, scatter, ntt)."""
    idx, w = _route_host(x, rW1, rb1, rW2, rb2)
    tok_of = [np.where((idx == e).any(-1))[0] for e in range(E)]
    w_of = [w[(t := tok_of[e]), (idx[t] == e).argmax(-1)]
            for e in range(E)]
    counts = np.array([len(t) for t in tok_of])
    C0, C1, plan = _plan(counts)

    xf = np.ascontiguousarray(np.asarray(x, np.float32).reshape(B, DIN))
    # [i, k, n] layout: xt[:, k, t] = xf[t, 128k + i]
    xt = xf.reshape(B, KC1, 128).transpose(2, 1, 0).astype(BF16)

    ew1_full = [np.asarray(eW1[e], np.float32)
                .reshape(KC1, 128, KC2, 128)
                .transpose(2, 1, 0, 3)
                .reshape(KC2, 128, DIN)
                .astype(BF16) for e in range(E)]
    eb1_full = [np.asarray(eb1[e], np.float32).reshape(KC2, 128).T
                for e in range(E)]
    ew2_full = [np.asarray(eW2[e], np.float32)
                .reshape(KC2, 128, NCLS)
                .transpose(1, 0, 2)
                .astype(BF16) for e in range(E)]

    pads = [-(-C0 // 128) * 128, -(-C1 // 128) * 128]
    ntt = (pads[0] + pads[1]) // 128
    caps = [C0, C1]

    in_maps = []
    scatter = []   # per core: (row_offset, token_ids) per slot
    for c in range(E):
        xblob = [np.zeros((128, KC1, C0), BF16),
                 np.zeros((128, KC1, C1), BF16)]
        ew1b = np.empty((KC2, 128, DIN), BF16)
        eb1b = np.empty((128, KC2), np.float32)
        ew2b = np.empty((128, KC2, NCLS), BF16)
        eb2b = np.empty((128, 2, NCLS), np.float32)
        wb = np.zeros((128, ntt), np.float32)
        half = c % 2
        sc = []
        for g in range(2):
            e = plan[c][g]
            toks = tok_of[e]
            n = len(toks)
            xblob[g][:, :, :n] = xt[:, :, toks]
            mlo = half * HK
            ew1b[g * HK:(g + 1) * HK] = ew1_full[e][mlo:mlo + HK]
            eb1b[:, g * HK:(g + 1) * HK] = eb1_full[e][:, mlo:mlo + HK]
            ew2b[:, g * HK:(g + 1) * HK] = ew2_full[e][:, mlo:mlo + HK]
            eb2b[:, g, :] = np.asarray(eb2[e], np.float32)[None, :]
            base_tt = 0 if g == 0 else pads[0] // 128
            wcol = np.zeros(pads[g], np.float32)
            wcol[:n] = w_of[e]
            wb[:, base_tt:base_tt + pads[g] // 128] = (
                wcol.reshape(-1, 128).T)
            j = np.arange(n)
            if g == 0:
                rows = base_tt * 128 + j
            else:
                # slot1 partials leave the core via one merged DMA with
                # rows interleaved as ntt1*token + tile
                ntt1 = pads[1] // 128
                rows = base_tt * 128 + (j % 128) * ntt1 + j // 128
            sc.append((rows, toks))
            assert n <= caps[g]
        in_maps.append({
            "x0": np.ascontiguousarray(xblob[0]),
            "x1": np.ascontiguousarray(xblob[1]),
            "ew1": np.ascontiguousarray(ew1b),
            "eb1": eb1b, "ew2": np.ascontiguousarray(ew2b),
            "eb2": eb2b, "w": wb,
        })
        scatter.append(sc)
    return in_maps, scatter, C0, C1


def kernel(x, rW1, rb1, rW2, rb2, eW1, eb1, eW2, eb2):
    global LAST_RESULTS
    _ensure_axon_profile_hook()
    from concourse.bass_utils import run_bass_kernel_spmd

    in_maps, scatter, C0, C1 = _prep_inputs(
        x, rW1, rb1, rW2, rb2, eW1, eb1, eW2, eb2)
    key = (C0, C1)
    nc = _PROGRAMS.get(key)
    if nc is None:
        nc = _build_program(C0, C1)
        nc.finalize()
        _PROGRAMS[key] = nc
    res = run_bass_kernel_spmd(nc, in_maps, core_ids=list(range(E)))
    LAST_RESULTS = res
    out = np.zeros((B, NCLS), np.float32)
    for c, r in enumerate(res.results):
        part = np.asarray(r["out"], np.float32)
        for (rows, toks) in scatter[c]:
            np.add.at(out, toks, part[rows])
    return out


# revision 21
# speedup vs baseline: 2.8027x; 1.0370x over previous
"""Trainium2 Bass kernel for the MoE routing module (nn_MoE_53042846105633).

Strategy: top-2 sparse dispatch, expert-parallel across 8 NeuronCores.
The router (0.8% of reference FLOPs) runs on host in fp64 during input
sharding -- the sharding_hint's "dispatch of tokens by top-k expert id"
-- which cuts device matmul work 4x vs dense: only the 2048 routed
(token, expert) pairs are computed instead of 8 experts x 1024 tokens.

Token counts per expert are skewed (e.g. [199,263,548,380,142,99,271,146]
for the seed-0 data), so each expert's 16 hidden tiles are split into two
8-tile halves and the 16 halves are packed onto cores as (big, small)
slot pairs: slot0 capacity C0 = largest expert count, slot1 capacity C1 =
5th-largest. All 8 cores run the identical instruction stream (SPMD);
per-core data (which expert half + which token columns) differs. With 16
half-slots and >=2 slots per expert this (C0 + C1) packing is optimal.

Per core: mm1 = 8 hid-tiles x 24 K-chunks over C0 cols + 8 x 24 over C1
(~60us PE at bf16), relu -> eh bf16, mm2 contracts eh against eW2 per
128-token stationary slice, bias + per-token routing weight (0 on padding)
-> weighted partial rows DMA'd out; host scatter-adds them into [B, 10].
Weights stream from HBM (12.6MB/core) well under the mm1 time. No
collectives (a NEFF with collectives drops the PE to 2.0 GHz). Expert
math is plain bf16 (fp32 PSUM accumulate), ~3e-3 relative output error;
the router/top-2/combine weights are exact (host fp64).
"""

import sys

sys.path.insert(0, "/opt/trn_rl_repo")

import numpy as np
import ml_dtypes

BF16 = ml_dtypes.bfloat16

# Model dims (fixed for this problem)
B = 1024          # tokens
DIN = 3072        # input features
RHID = 128        # router hidden
E = 8             # experts = cores
EHID = 2048       # expert hidden
NCLS = 10         # classes
KC1 = DIN // 128  # 24 K-chunks for DIN contraction
KC2 = EHID // 128 # 16 hid tiles per expert
HK = KC2 // 2     # 8 hid tiles per half-slot
TOPK = 2

_PROGRAMS = {}
LAST_RESULTS = None


def _ensure_axon_profile_hook():
    """bass_utils' trace=True path imports antenv.axon_hooks, which this
    image lacks. Provide it (backed by libaxon_pjrt.so's NRT profile C API)
    so NTFF profiling works; degrade silently if unavailable."""
    import contextlib
    import ctypes
    import os
    import types

    try:
        from antenv.axon_hooks import get_axon_ntff_profile_hook  # noqa: F401
        return
    except ImportError:
        pass
    try:
        import antenv
    except ImportError:
        return

    state = {"hook": None}
    mod = types.ModuleType("antenv.axon_hooks")
    mod.set_axon_ntff_profile_hook = lambda h: state.__setitem__("hook", h)
    mod.get_axon_ntff_profile_hook = lambda: state["hook"]
    sys.modules["antenv.axon_hooks"] = mod
    antenv.axon_hooks = mod

    so_path = "/opt/axon/libaxon_pjrt.so"
    if not os.path.exists(so_path):
        return
    try:
        lib = ctypes.CDLL(so_path)
    except OSError:
        return
    if not hasattr(lib, "axon_start_nrt_profile"):
        return
    lib.axon_start_nrt_profile.argtypes = [
        ctypes.POINTER(ctypes.c_int64), ctypes.c_size_t]
    lib.axon_start_nrt_profile.restype = ctypes.c_int64
    lib.axon_stop_nrt_profile.argtypes = [ctypes.c_char_p]
    lib.axon_stop_nrt_profile.restype = ctypes.c_int64

    @contextlib.contextmanager
    def _hook(output_dir, device_ids):
        import jax

        jax.devices()
        if device_ids:
            ids = (ctypes.c_int64 * len(device_ids))(*device_ids)
            rc = lib.axon_start_nrt_profile(ids, len(device_ids))
        else:
            rc = lib.axon_start_nrt_profile(None, 0)
        if rc != 0:
            raise RuntimeError(f"axon_start_nrt_profile rc={rc}")
        try:
            yield
        finally:
            n = lib.axon_stop_nrt_profile(str(output_dir).encode())
            print(f"profile: {n} ntff file(s) -> {output_dir}",
                  file=sys.stderr)

    state["hook"] = _hook


def _chunks(c):
    """Split c columns into <=512 PSUM-bank-sized pieces (each >=128 wide
    when possible so matmuls stay moving-bound)."""
    n = -(-c // 512)
    base = c // n
    out = []
    off = 0
    for i in range(n):
        w = base + (1 if i < c % n else 0)
        out.append((off, off + w))
        off += w
    return out


def _build_program(C0, C1):
    import concourse.tile as tile
    from concourse import bacc, mybir

    f32 = mybir.dt.float32
    bf = mybir.dt.bfloat16
    AF = mybir.ActivationFunctionType
    ALU = mybir.AluOpType

    caps = [C0] * HK + [C1] * HK      # columns per hid tile
    pads = [-(-C0 // 128) * 128, -(-C1 // 128) * 128]
    ntt = pads[0] // 128 + pads[1] // 128   # total 128-token out tiles
    RAMP = 4                          # slot0 tiles run k-outer in the ramp

    nc = bacc.Bacc("TRN2", debug=False, num_devices=E)

    # ---- DRAM I/O (all per-core data) ---------------------------------
    # x: [i, k, n]; slot0 tokens streamed per-k (paces the ramp k-outer),
    # slot1 tokens as one late transfer (first needed ~halfway through)
    d_x0 = nc.dram_tensor("x0", [128, KC1, C0], bf, kind="ExternalInput")
    d_x1 = nc.dram_tensor("x1", [128, KC1, C1], bf, kind="ExternalInput")
    # stationary tiles: t<8 slot0 expert half, t>=8 slot1 expert half
    # layout [t, i, (k j)]: element = eW1[e][128k + i, 128*m(t) + j]
    d_ew1 = nc.dram_tensor("ew1", [KC2, 128, DIN], bf, kind="ExternalInput")
    d_eb1 = nc.dram_tensor("eb1", [128, KC2], f32, kind="ExternalInput")
    # ew2[:, t, :] = eW2[e][128*m(t):128*(m(t)+1), :]
    d_ew2 = nc.dram_tensor("ew2", [128, KC2, NCLS], bf, kind="ExternalInput")
    # eb2 tiled to 128 partitions per slot
    d_eb2 = nc.dram_tensor("eb2", [128, 2, NCLS], f32, kind="ExternalInput")
    # per-token normalized routing weight, [token-in-tile, out-tile]
    d_w = nc.dram_tensor("w", [128, ntt], f32, kind="ExternalInput")
    # weighted partial rows; host scatter-adds real rows into [B, NCLS]
    d_out = nc.dram_tensor("out", [ntt * 128, NCLS], f32,
                           kind="ExternalOutput")

    with tile.TileContext(nc) as tc:
        with (
            tc.tile_pool(name="const", bufs=1) as cp,
            tc.tile_pool(name="wstream", bufs=6) as wp,
            tc.tile_pool(name="psum", bufs=1, space="PSUM") as pp,
            tc.tile_pool(name="outp", bufs=1) as op,
        ):
            # ---- HAM pre-warm: full-array K=128 dummies flip the PE
            # clock gate to 2.4 GHz during the DMA ramp (small-K matmuls
            # don't count as PE-busy).
            warmt = cp.tile([128, 128], bf, tag="warmt", name="warmt")
            nc.vector.memset(warmt[:], 1.0)
            warm = pp.tile([128, 128], f32, tag="mm1", bufs=8, name="warm")
            for _i in range(38):
                nc.tensor.matmul(warm[:], warmt[:], warmt[:],
                                 start=True, stop=True)

            # eh tiles, padded to 128-multiples; zero the pad columns so
            # mm2 stationary slices never read uninitialized SBUF.
            ehs = []
            for t in range(KC2):
                g = t // HK
                eh = cp.tile([128, pads[g]], bf, tag=f"eh{t}", name=f"eh{t}")
                ehs.append(eh)
                cap = caps[t]
                if cap < pads[g]:
                    nc.vector.memset(eh[:, cap:], 0.0)

            # ---- input DMA (emission order = FIFO queue order) --------
            # All 16 ew1 tiles are SBUF-resident (no WAR queue stalls);
            # slot0 x chunks pace the ramp k-outer; everything slot1
            # needs comes after the slot0-critical stream.
            wts = {}

            def load_ew1(t):
                wt = wp.tile([128, DIN], bf, tag="ew1", name=f"ew1t{t}")
                nc.sync.dma_start(wt[:, :DIN // 2], d_ew1[t][:, :DIN // 2])
                nc.sync.dma_start(wt[:, DIN // 2:], d_ew1[t][:, DIN // 2:])
                wts[t] = wt

            # Arrival model (300 GB/s sustained, first packet ~9.3us) used
            # to order the ramp's matmul emission by data readiness.
            arr = {}
            cum = [9300.0]

            def track(key, nbytes):
                cum[0] += nbytes / 0.30e3
                arr[key] = cum[0]

            eb1t = cp.tile([128, KC2], f32, tag="eb1", name="eb1t")
            nc.sync.dma_start(eb1t[:], d_eb1[:])
            track("eb1", KC2 * 512)
            x0k = []
            for k in range(KC1):
                t = cp.tile([128, C0], bf, tag=f"x0k{k}", name=f"x0k{k}")
                x0k.append(t)

            def load_x0(k):
                nc.sync.dma_start(x0k[k][:], d_x0[:, k, :])
                track(("x0", k), 128 * C0 * 2)

            # ramp tiles stream as halves with subtile deps: k<12 matmuls
            # only wait on the first half, so real work starts ~4us sooner
            def load_ew1_half(t, h):
                if t not in wts:
                    wts[t] = wp.tile([128, DIN], bf, tag="ew1",
                                     name=f"ew1t{t}")
                lo, hi = (0, DIN // 2) if h == 0 else (DIN // 2, DIN)
                nc.sync.dma_start(wts[t][:, lo:hi], d_ew1[t][:, lo:hi])
                track(("ew1", t, h), 128 * (DIN // 2) * 2)

            load_ew1_half(0, 0)
            load_x0(0)
            load_ew1_half(1, 0)
            load_x0(1)
            load_x0(2)
            load_ew1_half(2, 0)
            load_x0(3)
            load_x0(4)
            load_ew1_half(3, 0)
            for k in range(5, 13):
                load_x0(k)
            load_ew1_half(0, 1)
            load_x0(13)
            load_x0(14)
            load_ew1_half(1, 1)
            for k in range(15, 18):
                load_x0(k)
            load_ew1_half(2, 1)
            for k in range(18, 21):
                load_x0(k)
            load_ew1_half(3, 1)
            for k in range(21, KC1):
                load_x0(k)
            for t in range(4, HK):
                load_ew1(t)
            ew2t = cp.tile([128, KC2, NCLS], bf, tag="ew2", name="ew2t")
            nc.sync.dma_start(ew2t[:], d_ew2[:])
            eb2t = cp.tile([128, 2, NCLS], f32, tag="eb2", name="eb2t")
            nc.sync.dma_start(eb2t[:], d_eb2[:])
            wtok = cp.tile([128, ntt], f32, tag="wtok", name="wtok")
            nc.sync.dma_start(wtok[:], d_w[:])
            load_ew1(8)
            x1t = cp.tile([128, KC1, C1], bf, tag="x1t", name="x1t")
            nc.sync.dma_start(x1t[:], d_x1[:])
            for t in range(9, KC2):
                load_ew1(t)

            pos_g1 = {}

            def emit_mm2_g0():
                # out[tok, cls]: psum rows = tokens, stationary = 128-token
                # slices of this slot's eh tiles.
                for tt in range(pads[0] // 128):
                    po = pp.tile([128, NCLS], f32, tag="mm1", bufs=8,
                                 name=f"po0_{tt}")
                    for j in range(HK):
                        nc.tensor.matmul(
                            po[:],
                            ehs[j][:, tt * 128:(tt + 1) * 128],
                            ew2t[:, j, :],
                            start=(j == 0),
                            stop=(j == HK - 1),
                        )
                    osb = op.tile([128, NCLS], f32, tag="osb", bufs=4,
                                  name=f"osb0_{tt}")
                    nc.vector.tensor_add(osb[:], po[:], eb2t[:, 0, :])
                    nc.vector.tensor_scalar(
                        osb[:], osb[:], wtok[:, tt:tt + 1], None, ALU.mult)
                    nc.sync.dma_start(
                        d_out[tt * 128:(tt + 1) * 128, :], osb[:])

            def emit_mm2_g1(js, finish):
                # slot1 mm2, split so only `js` of the contraction runs
                # after the last mm1 tile; one merged out DMA at the end
                # (rows interleaved as base + ntt1*token + tt; the host
                # scatter indexes accordingly).
                base = pads[0] // 128
                ntt1 = pads[1] // 128
                for tt in range(ntt1):
                    po = pos_g1.get(tt)
                    if po is None:
                        po = pp.tile([128, NCLS], f32, tag="mm1", bufs=8,
                                     name=f"po1_{tt}")
                        pos_g1[tt] = po
                    for j in js:
                        nc.tensor.matmul(
                            po[:],
                            ehs[HK + j][:, tt * 128:(tt + 1) * 128],
                            ew2t[:, HK + j, :],
                            start=(j == 0),
                            stop=(finish and j == js[-1]),
                        )
                if not finish:
                    return
                osb1 = op.tile([128, ntt1, NCLS], f32, tag="osb1",
                               name="osb1")
                for tt in range(ntt1):
                    nc.vector.tensor_add(osb1[:, tt, :], pos_g1[tt][:],
                                         eb2t[:, 1, :])
                    nc.vector.tensor_scalar(
                        osb1[:, tt, :], osb1[:, tt, :],
                        wtok[:, base + tt:base + tt + 1], None, ALU.mult)
                nc.sync.dma_start(d_out[base * 128:, :], osb1[:])

            # ---- mm1: eh[t] = relu(W1_tile[t].T @ x_cols + b) ----------
            def xsrc(t, k, a, b):
                return (x0k[k][:, a:b] if t < HK
                        else x1t[:, k, a:b])

            ch0 = _chunks(C0)
            # ramp: first RAMP slot0 tiles jointly k-outer; cells (t, k)
            # emitted in predicted-data-arrival order so the PE never
            # idles (idle gaps also drop the HAM clock below 2.4 GHz)
            ramp_pss = {t: [pp.tile([128, b - a], f32, tag="mm1", bufs=8,
                                    name=f"ps1_{t}_{i}")
                            for i, (a, b) in enumerate(ch0)]
                        for t in range(RAMP)}
            cells = sorted(
                ((max(arr[("ew1", t, 0 if k < KC1 // 2 else 1)],
                      arr[("x0", k)]), k, t)
                 for t in range(RAMP) for k in range(KC1)),
                key=lambda c: (c[0], c[1], c[2]))
            seen = {t: 0 for t in range(RAMP)}
            for _, k, t in cells:
                seen[t] += 1
                for i, (a, b) in enumerate(ch0):
                    nc.tensor.matmul(
                        ramp_pss[t][i][:],
                        wts[t][:, k * 128:(k + 1) * 128],
                        x0k[k][:, a:b],
                        start=(seen[t] == 1),
                        stop=(seen[t] == KC1),
                    )
            for t in range(RAMP):
                for i, (a, b) in enumerate(ch0):
                    nc.scalar.activation(
                        ehs[t][:, a:b], ramp_pss[t][i][:],
                        AF.Relu, bias=eb1t[:, t:t + 1],
                    )

            for t in range(RAMP, KC2):
                wt = wts[t]
                cap = caps[t]
                ch = _chunks(cap)
                pss = [pp.tile([128, b - a], f32, tag="mm1", bufs=8,
                               name=f"ps1_{t}_{i}") for i, (a, b) in
                       enumerate(ch)]
                for i, (a, b) in enumerate(ch):
                    for k in range(KC1):
                        nc.tensor.matmul(
                            pss[i][:],
                            wt[:, k * 128:(k + 1) * 128],
                            xsrc(t, k, a, b),
                            start=(k == 0),
                            stop=(k == KC1 - 1),
                        )
                for i, (a, b) in enumerate(ch):
                    nc.scalar.activation(
                        ehs[t][:, a:b], pss[i][:],
                        AF.Relu, bias=eb1t[:, t:t + 1],
                    )
                if t == HK + 1:
                    # slot0's eh tiles are complete; its mm2 hides under
                    # the remaining slot1 mm1 work
                    emit_mm2_g0()
                if t == KC2 - 5:
                    # pre-accumulate the first half of slot1's mm2 so
                    # only 4 matmuls per out-tile trail the last mm1 tile
                    emit_mm2_g1(list(range(4)), finish=False)
            emit_mm2_g1(list(range(4, HK)), finish=True)

    return nc


def _route_host(x, rW1, rb1, rW2, rb2):
    """Exact router on host: top-2 expert ids + normalized weights."""
    xf = np.asarray(x, np.float64).reshape(B, DIN)
    rh = np.maximum(xf @ np.asarray(rW1, np.float64)
                    + np.asarray(rb1, np.float64), 0.0)
    logits = rh @ np.asarray(rW2, np.float64) + np.asarray(rb2, np.float64)
    m = logits.max(-1, keepdims=True)
    p = np.exp(logits - m)
    p /= p.sum(-1, keepdims=True)
    idx = np.argsort(-p, axis=-1, kind="stable")[:, :TOPK]
    w = np.take_along_axis(p, idx, axis=-1)
    w /= w.sum(-1, keepdims=True)
    return idx, w


def _plan(counts):
    """Pack the 16 expert-halves into 8 (slot0, slot1) core pairs.

    Returns (C0, C1, plan) with plan[c] = [expert_slot0, expert_slot1];
    core c takes half c%2 of each. Slot0 = the 4 largest experts, slot1 =
    the 4 smallest: provably minimal C0 + C1 for this slot structure.
    """
    order = np.argsort(-counts, kind="stable")
    C0 = int(counts[order[0]])
    C1 = max(int(counts[order[4]]), 1)
    plan = []
    for c in range(E):
        plan.append((int(order[c // 2]), int(order[4 + c // 2])))
    return C0, C1, plan


def _prep_inputs(x, rW1, rb1, rW2, rb2, eW1, eb1, eW2, eb2):
    """Host routing + shard/layout prep. Returns (in_maps, scatter, ntt)."""
    idx, w = _route_host(x, rW1, rb1, rW2, rb2)
    tok_of = [np.where((idx == e).any(-1))[0] for e in range(E)]
    w_of = [w[(t := tok_of[e]), (idx[t] == e).argmax(-1)]
            for e in range(E)]
    counts = np.array([len(t) for t in tok_of])
    C0, C1, plan = _plan(counts)

    xf = np.ascontiguousarray(np.asarray(x, np.float32).reshape(B, DIN))
    # [i, k, n] layout: xt[:, k, t] = xf[t, 128k + i]
    xt = xf.reshape(B, KC1, 128).transpose(2, 1, 0).astype(BF16)

    ew1_full = [np.asarray(eW1[e], np.float32)
                .reshape(KC1, 128, KC2, 128)
                .transpose(2, 1, 0, 3)
                .reshape(KC2, 128, DIN)
                .astype(BF16) for e in range(E)]
    eb1_full = [np.asarray(eb1[e], np.float32).reshape(KC2, 128).T
                for e in range(E)]
    ew2_full = [np.asarray(eW2[e], np.float32)
                .reshape(KC2, 128, NCLS)
                .transpose(1, 0, 2)
                .astype(BF16) for e in range(E)]

    pads = [-(-C0 // 128) * 128, -(-C1 // 128) * 128]
    ntt = (pads[0] + pads[1]) // 128
    caps = [C0, C1]

    in_maps = []
    scatter = []   # per core: (row_offset, token_ids) per slot
    for c in range(E):
        xblob = [np.zeros((128, KC1, C0), BF16),
                 np.zeros((128, KC1, C1), BF16)]
        ew1b = np.empty((KC2, 128, DIN), BF16)
        eb1b = np.empty((128, KC2), np.float32)
        ew2b = np.empty((128, KC2, NCLS), BF16)
        eb2b = np.empty((128, 2, NCLS), np.float32)
        wb = np.zeros((128, ntt), np.float32)
        half = c % 2
        sc = []
        for g in range(2):
            e = plan[c][g]
            toks = tok_of[e]
            n = len(toks)
            xblob[g][:, :, :n] = xt[:, :, toks]
            mlo = half * HK
            ew1b[g * HK:(g + 1) * HK] = ew1_full[e][mlo:mlo + HK]
            eb1b[:, g * HK:(g + 1) * HK] = eb1_full[e][:, mlo:mlo + HK]
            ew2b[:, g * HK:(g + 1) * HK] = ew2_full[e][:, mlo:mlo + HK]
            eb2b[:, g, :] = np.asarray(eb2[e], np.float32)[None, :]
            base_tt = 0 if g == 0 else pads[0] // 128
            wcol = np.zeros(pads[g], np.float32)
            wcol[:n] = w_of[e]
            wb[:, base_tt:base_tt + pads[g] // 128] = (
                wcol.reshape(-1, 128).T)
            j = np.arange(n)
            if g == 0:
                rows = base_tt * 128 + j
            else:
                # slot1 partials leave the core via one merged DMA with
                # rows interleaved as ntt1*token + tile
                ntt1 = pads[1] // 128
                rows = base_tt * 128 + (j % 128) * ntt1 + j // 128
            sc.append((rows, toks))
            assert n <= caps[g]
        in_maps.append({
            "x0": np.ascontiguousarray(xblob[0]),
            "x1": np.ascontiguousarray(xblob[1]),
            "ew1": np.ascontiguousarray(ew1b),
            "eb1": eb1b, "ew2": np.ascontiguousarray(ew2b),
            "eb2": eb2b, "w": wb,
        })
        scatter.append(sc)
    return in_maps, scatter, C0, C1


def kernel(x, rW1, rb1, rW2, rb2, eW1, eb1, eW2, eb2):
    global LAST_RESULTS
    _ensure_axon_profile_hook()
    from concourse.bass_utils import run_bass_kernel_spmd

    in_maps, scatter, C0, C1 = _prep_inputs(
        x, rW1, rb1, rW2, rb2, eW1, eb1, eW2, eb2)
    key = (C0, C1)
    nc = _PROGRAMS.get(key)
    if nc is None:
        nc = _build_program(C0, C1)
        nc.finalize()
        _PROGRAMS[key] = nc
    res = run_bass_kernel_spmd(nc, in_maps, core_ids=list(range(E)))
    LAST_RESULTS = res
    out = np.zeros((B, NCLS), np.float32)
    for c, r in enumerate(res.results):
        part = np.asarray(r["out"], np.float32)
        for (rows, toks) in scatter[c]:
            np.add.at(out, toks, part[rows])
    return out
